# revision 36
# baseline (speedup 1.0000x reference)
"""BitNet-style attention layer (B=2, T=2048, D=1024, 16 heads, RoPE, causal)
on 8 TRN2 NeuronCores.

Sharding: head-parallel attention (2 heads/core); wo is computed per-core for
an o-slice after an AllGather of the int8-valued (bf16-stored) quantized
attention output.  Per-token output-quant scales come from a per-batch
AllReduce(max) pipelined behind the attention of the other batch.

Pipeline layout (single fused graph):
  A: per 512-token block: DMA x, quantize (scalar+vector), PE-transpose to
     f32r, QKV projections, in-place RoPE, V-transpose (+ones column for
     fused softmax row-sums).
  B: attention as a flat 2-ahead software-pipelined stream per batch; causal
     masking is a 0/1 multiply on A (vector) so the PE never touches masks;
     the epilogue (amax -> AllReduce -> quantize -> AllGather -> wo) runs
     per batch, hooked into the other batch's stream.
"""

import math
from contextlib import ExitStack

import ml_dtypes
import numpy as np

import concourse.bass as bass
import concourse.bacc as bacc_mod
import concourse.mybir as mybir
import concourse.tile as tile
from concourse.bass_utils import run_bass_kernel_spmd

F32 = mybir.dt.float32
F32R = mybir.dt.float32r
BF16 = mybir.dt.bfloat16
OP = mybir.AluOpType
ACT = mybir.ActivationFunctionType

B, T, D = 2, 2048, 1024
NT = B * T              # 4096 tokens
NH, HD = 16, 64
HDP1 = HD + 1           # V augmented with a ones column (fused row-sum)
N_CORES = 8
HPC = NH // N_CORES     # heads per core = 2
DPC = HPC * HD          # dims per core = 128
RC = 12582912.0         # 1.5*2^23: round-to-nearest-even constant

TB = 512                # token block (matmul N; HW moving-dim max)
NTB = NT // TB          # 8
NTT = NT // 128         # 32 token tiles
QB = 512                # q block
NQB = T // QB           # 4 per batch
NKT = T // 128          # 16 k tiles per batch
TTPB = T // 128         # 16 token tiles per batch


def _quant_w(w):
    O, I = w.shape
    wg = w.reshape(O, I // 128, 128)
    ws = np.abs(wg).mean(-1, keepdims=True) + 1e-5
    wq = np.clip(np.round(wg / ws), -1.0, 1.0) * ws
    return wq.reshape(O, I).astype(np.float32)


def build_nc():
    nc = bacc_mod.Bacc(num_devices=N_CORES)
    io = {}

    def inp(name, shape, dt=F32):
        io[name] = nc.dram_tensor(name, shape, dt, kind="ExternalInput")

    inp("x", [NT, D])
    inp("sxp", [128, NTT])
    inp("isx", [128, NT])
    inp("wqT", [D, DPC], F32R)
    inp("wkT", [D, DPC], F32R)
    inp("wvT", [D, DPC], F32R)
    inp("woT", [D, DPC], BF16)
    inp("cmap", [128, NT])
    inp("smap", [128, NT])
    inp("pswapT", [128, 128], F32R)
    inp("umask01", [128, 4 * 2 * QB], BF16)   # keep-mask, both head halves
    inp("sel2", [33, 128], F32R)
    inp("ones1", [1, 128], F32R)
    inp("identR", [128, 128], F32R)
    inp("identB", [128, 128], BF16)
    out = nc.dram_tensor("out", [DPC, NT], F32, kind="ExternalOutput")

    r32 = lambda ap: ap.bitcast(F32R)
    RG = [list(range(N_CORES))]

    with nc.allow_low_precision(reason="f32r matmul pipeline (FP22 mantissa is ample here)"), \
         tile.TileContext(nc) as tc, ExitStack() as top:
        cpool = top.enter_context(tc.tile_pool(name="const", bufs=1))
        dpool = top.enter_context(tc.tile_pool(name="dram", bufs=1, space="DRAM"))

        # ---- constants (hot ones first on sync; the rest on gpsimd, in
        # approximate order of first use, so phase A starts immediately)
        def const_tile(name, shape, dt=F32, src=None, q=None):
            t = cpool.tile(shape, dt, tag=name)
            (q or nc.gpsimd).dma_start(t[:], src if src is not None
                                       else io[name][:])
            return t

        sxp = const_tile("sxp", [128, NTT], q=nc.sync)
        identB = const_tile("identB", [128, 128], BF16, q=nc.sync)
        wq_sb = [const_tile(f"wq{i}", [128, DPC], F32R, io["wqT"][i * 128:(i + 1) * 128, :]) for i in range(8)]
        wk_sb = [const_tile(f"wk{i}", [128, DPC], F32R, io["wkT"][i * 128:(i + 1) * 128, :]) for i in range(8)]
        wv_sb = [const_tile(f"wv{i}", [128, DPC], F32R, io["wvT"][i * 128:(i + 1) * 128, :]) for i in range(8)]
        pswapT = const_tile("pswapT", [128, 128], F32R)
        identR = const_tile("identR", [128, 128], F32R)
        umask01 = const_tile("umask01", [128, 4 * 2 * QB], BF16)
        sel2 = const_tile("sel2", [33, 128], F32R)
        ones1 = const_tile("ones1", [1, 128], F32R)
        wo_sb = [const_tile(f"wo{i}", [128, DPC], BF16, io["woT"][i * 128:(i + 1) * 128, :]) for i in range(8)]

        # ---- persistent SBUF state
        es_qk = ExitStack()
        qkp = es_qk.enter_context(tc.tile_pool(name="qk", bufs=1))
        qT = qkp.tile([128, NT], F32R, name="qT", tag="qT")
        kT = qkp.tile([128, NT], F32R, name="kT", tag="kT")
        es_va = ExitStack()
        vap = es_va.enter_context(tc.tile_pool(name="vap", bufs=1))
        vaug = [[None] * NTT for _ in range(HPC)]
        es_v = ExitStack()
        vp = es_v.enter_context(tc.tile_pool(name="vp", bufs=1))
        vT = vp.tile([128, NT], F32R, name="vT", tag="vT")

        # ---- per-batch / per-segment DRAM collective buffers
        ar_in = [dpool.tile([128, TTPB], F32, name=f"ar_in{b}", tag=f"ar_in{b}")
                 for b in range(B)]
        ar_out = [dpool.tile([128, TTPB], F32, name=f"ar_out{b}",
                             tag=f"ar_out{b}", addr_space="Shared")
                  for b in range(B)]
        # AG segments: (batch, tok_start, n_tok); b1 split for tail overlap
        AGSEG = [(0, 0, 2048), (1, 2048, 1536), (1, 3584, 512)]
        ag_in = [dpool.tile([128, n], BF16, name=f"ag_in{s}", tag=f"ag_in{s}")
                 for s, (_, _, n) in enumerate(AGSEG)]
        ag_out = [dpool.tile([N_CORES * 128, n], BF16, name=f"ag_out{s}",
                             tag=f"ag_out{s}", addr_space="Shared")
                  for s, (_, _, n) in enumerate(AGSEG)]

        # ======== Phase A: quantize x, transpose, QKV proj, RoPE, V-transpose
        with tc.tile_pool(name="pxt", bufs=4) as pxt, \
             tc.tile_pool(name="pxf", bufs=2) as pxf, \
             tc.tile_pool(name="pm", bufs=2) as pm, \
             tc.tile_pool(name="ptmp", bufs=2) as ptmp, \
             tc.tile_pool(name="ptp", bufs=2, space="PSUM") as ptp, \
             tc.tile_pool(name="ppp", bufs=2, space="PSUM") as ppp, \
             tc.tile_pool(name="prp", bufs=2, space="PSUM") as prp, \
             tc.tile_pool(name="pvt", bufs=2, space="PSUM") as pvt:
            for tb in range(NTB):
                sl = slice(tb * TB, (tb + 1) * TB)
                xf = [pxf.tile([128, TB], F32R, name=f"xf{i}", tag=f"xf{i}")
                      for i in range(8)]
                for lt in range(4):
                    tt = tb * 4 + lt
                    xt = pxt.tile([128, D], F32, name="xt", tag="xt")
                    nc.sync.dma_start(xt[:], io["x"][tt * 128:(tt + 1) * 128, :])
                    y = pxt.tile([128, D], F32, name="y", tag="y")
                    nc.scalar.activation(y[:], xt[:], ACT.Copy, bias=RC,
                                         scale=sxp[:, tt:tt + 1])
                    xi = pxt.tile([128, D], BF16, name="xi", tag="xi")
                    nc.vector.tensor_scalar(xi[:], y[:], RC, None, OP.subtract)
                    for i in range(8):
                        tp = ptp.tile([128, 128], BF16, name="tp", tag="tp")
                        nc.tensor.transpose(tp[:], xi[:, i * 128:(i + 1) * 128],
                                            identB[:])
                        dst = xf[i][:, lt * 128:(lt + 1) * 128]
                        if i < 4:
                            nc.scalar.copy(dst, tp[:])
                        else:
                            nc.vector.tensor_copy(dst, tp[:])
                # projections
                isxb = pm.tile([128, TB], F32, name="isxb", tag="isxb")
                nc.sync.dma_start(isxb[:], io["isx"][:, sl])
                cm = pm.tile([128, TB], F32, name="cm", tag="cm")
                nc.sync.dma_start(cm[:], io["cmap"][:, sl])
                sm = pm.tile([128, TB], F32, name="sm", tag="sm")
                nc.sync.dma_start(sm[:], io["smap"][:, sl])
                for w_sb, dstT in ((wq_sb, qT), (wk_sb, kT), (wv_sb, vT)):
                    pp = ppp.tile([128, TB], F32, name="pp", tag="pp")
                    for i in range(8):
                        nc.tensor.matmul(pp[:], w_sb[i][:], xf[i][:],
                                         start=(i == 0), stop=(i == 7))
                    nc.vector.tensor_tensor(dstT[:, sl], pp[:], isxb[:],
                                            OP.mult)
                # RoPE in place on q, k
                for srcT in (qT, kT):
                    swp = prp.tile([128, TB], F32, name="swp", tag="swp")
                    nc.tensor.matmul(swp[:], pswapT[:], srcT[:, sl],
                                     start=True, stop=True)
                    tmp = ptmp.tile([128, TB], F32, name="tmp", tag="tmp")
                    nc.gpsimd.tensor_tensor(tmp[:], srcT[:, sl], cm[:], OP.mult)
                    tmp2 = ptmp.tile([128, TB], F32, name="tmp2", tag="tmp2")
                    nc.vector.tensor_tensor(tmp2[:], swp[:], sm[:], OP.mult)
                    nc.gpsimd.tensor_tensor(srcT[:, sl], tmp[:], tmp2[:], OP.add)
                # V transpose (+ones column) for this block's 4 k-tiles
                for lt in range(4):
                    kt = tb * 4 + lt
                    vtp = pvt.tile([128, 128], F32, name="vtp", tag="vtp")
                    nc.tensor.transpose(r32(vtp[:]),
                                        vT[:, kt * 128:(kt + 1) * 128],
                                        identR[:])
                    va0 = vap.tile([128, HDP1], BF16, name=f"va0_{kt}",
                                   tag=f"va0_{kt}")
                    nc.vector.memset(va0[:, HD:HDP1], 1.0)
                    nc.scalar.copy(va0[:, 0:HD], vtp[:, 0:HD])
                    va1 = vap.tile([128, HDP1], BF16, name=f"va1_{kt}",
                                   tag=f"va1_{kt}")
                    nc.vector.memset(va1[:, HD:HDP1], 1.0)
                    nc.vector.tensor_copy(va1[:, 0:HD], vtp[:, HD:128])
                    vaug[0][kt] = va0
                    vaug[1][kt] = va1
        es_v.close()

        # ======== Phase B: attention with per-batch pipelined epilogue
        es_b = ExitStack()
        big = es_b.enter_context(tc.tile_pool(name="big", bufs=1))
        out_n = big.tile([128, NT], F32R, name="out_n", tag="out_n")
        xio = big.tile([128, NT], BF16, name="xio", tag="xio")
        rsi = big.tile([33, QB], F32, name="rsi", tag="rsi")
        rsr = big.tile([33, QB], F32R, name="rsr", tag="rsr")
        nc.vector.memset(rsi[:], 1.0)
        prow = es_b.enter_context(tc.tile_pool(name="prow", bufs=2))
        so_rows = [None] * B
        iso_rows = [None] * B
        psSp = es_b.enter_context(tc.tile_pool(name="psS", bufs=3, space="PSUM"))
        pAcc = es_b.enter_context(tc.tile_pool(name="pAcc", bufs=1, space="PSUM"))
        pA = es_b.enter_context(tc.tile_pool(name="pA", bufs=3))
        pscl = es_b.enter_context(tc.tile_pool(name="pscl", bufs=2))
        pyq = es_b.enter_context(tc.tile_pool(name="pyq", bufs=2))
        pg = es_b.enter_context(tc.tile_pool(name="pg", bufs=2))
        pfin = es_b.enter_context(tc.tile_pool(name="pfin", bufs=2))

        def post_ps():
            # post-work PSUM tiles share the psS pool (PSUM is fully budgeted)
            return psSp.tile([128, 2 * QB], F32, name="psS", tag="psS")

        def attn_batch(b, hooks=None):
            """Whole-batch attention as one flat software-pipelined stream:
            scores run 2 (qb, kl)-stages ahead of A@V, so neither the exp
            latency nor the per-qb epilogue ever starves the PE queue.
            hooks[i] is emitted after pipeline step i (mid-batch interleave
            of the other batch's post-processing)."""
            seq = [(qb, kl) for qb in range(NQB) for kl in range(4 * qb + 4)]
            n = len(seq)
            psA = {}
            psS_l = {}
            A_l = {}

            def scores(i):
                qb, kl = seq[i]
                # diag band tile v: queries < v*128 are fully masked — skip
                off = max(0, kl - 4 * qb) * 128
                qsl = slice(b * T + qb * QB + off, b * T + (qb + 1) * QB)
                kt = b * NKT + kl
                ksl = slice(kt * 128, (kt + 1) * 128)
                psS = psSp.tile([128, 2 * QB], F32, name="psS", tag="psS")
                for h in range(HPC):
                    hsl = slice(h * HD, (h + 1) * HD)
                    ssl = slice(h * QB + off, (h + 1) * QB)
                    nc.tensor.matmul(psS[:, ssl], kT[hsl, ksl], qT[hsl, qsl],
                                     start=True, stop=True)
                psS_l[i] = psS

            def do_exp(i):
                qb, kl = seq[i]
                A = pA.tile([128, 2 * QB], BF16, name="A", tag="A")
                nc.scalar.activation(A[:], psS_l.pop(i)[:], ACT.Exp,
                                     scale=1.0 / math.sqrt(HD))
                if kl >= 4 * qb:     # diagonal band: zero the future keys
                    v = kl - 4 * qb
                    nc.vector.tensor_tensor(
                        A[:], A[:], umask01[:, v * 2 * QB:(v + 1) * 2 * QB],
                        OP.mult)
                A_l[i] = A

            def av(i):
                qb, kl = seq[i]
                nkt = 4 * qb + 4
                if kl == 0:
                    psA[qb] = (pAcc.tile([128, QB], F32, name="psA0", tag="psA0"),
                               pAcc.tile([128, QB], F32, name="psA1", tag="psA1"))
                psA0, psA1 = psA[qb]
                kt = b * NKT + kl
                A = A_l.pop(i)
                off = max(0, kl - 4 * qb) * 128
                st, sp = kl == 0, kl == nkt - 1
                nc.tensor.matmul(psA0[0:HDP1, off:QB], vaug[0][kt][:],
                                 A[:, off:QB], start=st, stop=sp,
                                 skip_group_check=True)
                nc.tensor.matmul(psA1[0:HDP1, off:QB], vaug[1][kt][:],
                                 A[:, QB + off:2 * QB], start=st, stop=sp,
                                 skip_group_check=True)
                if sp:
                    epilogue(qb, psA.pop(qb))

            def epilogue(qb, psA01):
                psA0, psA1 = psA01
                qsl = slice(b * T + qb * QB, b * T + (qb + 1) * QB)
                # fused row-sums live at psA0[HD], psA1[HD]
                nc.vector.tensor_copy(rsi[0:1, :], psA0[HD:HDP1, :])
                nc.vector.tensor_copy(rsi[32:33, :], psA1[HD:HDP1, :])
                nc.vector.reciprocal(rsr[:], rsi[:])
                brs = post_ps()
                nc.tensor.matmul(brs[:, 0:QB], sel2[:], rsr[:], start=True,
                                 stop=True)
                brs_sb = pyq.tile([128, QB], F32, name="brs_sb", tag="brs_sb")
                nc.vector.tensor_copy(brs_sb[:], brs[:, 0:QB])
                nc.vector.tensor_tensor(out_n[0:HD, qsl], psA0[0:HD, :],
                                        brs_sb[0:HD, :], OP.mult)
                nc.vector.tensor_tensor(out_n[HD:128, qsl], psA1[0:HD, :],
                                        brs_sb[HD:128, :], OP.mult)

            scores(0)
            scores(1)
            do_exp(0)
            for i in range(2, n):
                scores(i)
                do_exp(i - 1)
                av(i - 2)
                if hooks and i in hooks:
                    hooks[i]()
            do_exp(n - 1)
            av(n - 2)
            av(n - 1)

        def amax_ar(b):
            am = pscl.tile([128, TTPB], F32, name="am", tag="am")
            for j in range(TTPB):
                gtt = b * TTPB + j
                tp = post_ps()
                nc.tensor.transpose(r32(tp[:, 0:128]),
                                    out_n[:, gtt * 128:(gtt + 1) * 128],
                                    identR[:])
                nc.vector.tensor_reduce(am[:, j:j + 1], tp[:, 0:128],
                                        mybir.AxisListType.X, OP.max,
                                        apply_absolute_value=True)
            nc.sync.dma_start(ar_in[b][:], am[:])
            nc.gpsimd.collective_compute(
                "AllReduce", OP.max, replica_groups=RG,
                ins=[ar_in[b][:].opt()], outs=[ar_out[b][:].opt()])

        def scale_rows(b):
            gmax = pscl.tile([128, TTPB], F32, name="gmax", tag="gmax")
            nc.sync.dma_start(gmax[:], ar_out[b][:])
            iso_p = pscl.tile([128, TTPB], F32R, name="iso_p", tag="iso_p")
            nc.vector.tensor_scalar(iso_p[:], gmax[:], 1e-5, 1.0 / 127.0,
                                    OP.add, OP.mult)
            so_p = pscl.tile([128, TTPB], F32R, name="so_p", tag="so_p")
            nc.vector.reciprocal(so_p[:], iso_p[:])
            tso = post_ps()
            nc.tensor.transpose(r32(tso[0:TTPB, 0:128]), so_p[:], identR[:])
            so_sq = pscl.tile([TTPB, 128], F32R, name="so_sq", tag="so_sq")
            nc.vector.tensor_copy(so_sq[:], tso[0:TTPB, 0:128])
            tiso = post_ps()
            nc.tensor.transpose(r32(tiso[0:TTPB, 0:128]), iso_p[:], identR[:])
            iso_sq = pscl.tile([TTPB, 128], F32R, name="iso_sq", tag="iso_sq")
            nc.vector.tensor_copy(iso_sq[:], tiso[0:TTPB, 0:128])
            so_rows[b] = prow.tile([1, T], F32R, name="so_row", tag="so_row")
            iso_rows[b] = prow.tile([1, T], F32R, name="iso_row", tag="iso_row")
            nc.sync.dma_start(so_rows[b][0:1, :], so_sq[:, :])
            nc.sync.dma_start(iso_rows[b][0:1, :], iso_sq[:, :])

        def quant_ag(seg):
            b, t0, ntok = AGSEG[seg]
            for tl in range(ntok // TB):
                sl = slice(t0 + tl * TB, t0 + (tl + 1) * TB)
                rsl = slice(t0 - b * T + tl * TB, t0 - b * T + (tl + 1) * TB)
                bso = post_ps()
                nc.tensor.matmul(bso[:, 0:QB], ones1[:], so_rows[b][:, rsl],
                                 start=True, stop=True)
                yq = pyq.tile([128, TB], F32, name="yq", tag="yq")
                nc.vector.tensor_tensor(yq[:], out_n[:, sl], bso[:, 0:QB],
                                        OP.mult)
                nc.vector.tensor_scalar(xio[:, sl], yq[:], RC, RC,
                                        OP.add, OP.subtract)
            nc.sync.dma_start(ag_in[seg][:], xio[:, t0:t0 + ntok])
            nc.gpsimd.collective_compute(
                "AllGather", OP.bypass, replica_groups=RG,
                ins=[ag_in[seg][:].opt()], outs=[ag_out[seg][:].opt()])

        def wo(seg):
            b, t0, ntok = AGSEG[seg]
            for tl in range(ntok // TB):
                sl = slice(t0 + tl * TB, t0 + (tl + 1) * TB)
                rsl = slice(t0 - b * T + tl * TB, t0 - b * T + (tl + 1) * TB)
                pw = post_ps()
                for i in range(8):
                    g = pg.tile([128, TB], BF16, name=f"g{i}", tag=f"g{i}")
                    nc.sync.dma_start(g[:], ag_out[seg][i * 128:(i + 1) * 128,
                                                       tl * TB:(tl + 1) * TB])
                    nc.tensor.matmul(pw[:, 0:QB], wo_sb[i][:], g[:],
                                     start=(i == 0), stop=(i == 7))
                bi = post_ps()
                nc.tensor.matmul(bi[:, 0:QB], ones1[:], iso_rows[b][:, rsl],
                                 start=True, stop=True)
                bi_sb = pyq.tile([128, TB], F32, name="bi_sb", tag="bi_sb")
                nc.scalar.copy(bi_sb[:], bi[:, 0:QB])
                fin = pfin.tile([128, TB], F32, name="fin", tag="fin")
                nc.vector.tensor_tensor(fin[:], pw[:, 0:QB], bi_sb[:], OP.mult)
                nc.sync.dma_start(out[:, sl], fin[:])

        def post_b0():
            scale_rows(0)
            quant_ag(0)

        attn_batch(0)
        amax_ar(0)
        attn_batch(1, hooks={18: post_b0})
        amax_ar(1)
        wo(0)
        scale_rows(1)
        quant_ag(1)
        quant_ag(2)
        wo(1)
        wo(2)

        es_b.close()
        es_va.close()
        es_qk.close()

    return nc


_CACHE = {}


def kernel(x, cos, sin, wq_w, wk_w, wv_w, wo_w):
    x = np.asarray(x, np.float32)
    cos = np.asarray(cos, np.float32)   # [T, 32]
    sin = np.asarray(sin, np.float32)
    xf = np.ascontiguousarray(x.reshape(NT, D))

    amax = np.abs(xf).max(-1) + 1e-5
    sx = (127.0 / amax).astype(np.float32)
    isx = (amax / 127.0).astype(np.float32)
    sxp = np.ascontiguousarray(sx.reshape(NTT, 128).T)
    isx_bc = np.ascontiguousarray(np.broadcast_to(isx[None, :], (128, NT)))

    # RoPE maps from the provided cos/sin tables
    cm64 = np.repeat(cos.T, 2, axis=0)            # [64, T]
    sm64 = np.repeat(sin.T, 2, axis=0)
    # rows: [64 dims for head-even][64 dims for head-odd]; cols: [b0 | b1]
    cmap = np.tile(np.concatenate([cm64, cm64], axis=0), (1, B)).astype(np.float32)
    smap = np.tile(np.concatenate([sm64, sm64], axis=0), (1, B)).astype(np.float32)

    P = np.zeros((128, 128), np.float32)
    for j in range(64):
        P[2 * j, 2 * j + 1] = -1.0
        P[2 * j + 1, 2 * j] = 1.0
    pswapT = np.ascontiguousarray(P.T)
    kk = np.arange(128)[:, None]
    qq = np.arange(QB)[None, :]
    # keep-mask (1 = attend, 0 = future), duplicated for both head halves
    um1 = [((v * 128 + kk) <= qq).astype(np.float32) for v in range(4)]
    um01 = np.concatenate([np.concatenate([m, m], axis=1) for m in um1],
                          axis=1).astype(ml_dtypes.bfloat16)
    sel2 = np.zeros((33, 128), np.float32)
    sel2[0, 0:HD] = 1.0
    sel2[32, HD:128] = 1.0
    ones1 = np.ones((1, 128), np.float32)
    ident = np.eye(128, dtype=np.float32)
    identB = np.eye(128, dtype=np.float32).astype(ml_dtypes.bfloat16)

    wq_e, wk_e, wv_e, wo_e = (_quant_w(np.asarray(w, np.float32))
                              for w in (wq_w, wk_w, wv_w, wo_w))

    if "nc" not in _CACHE:
        nc0 = build_nc()
        nc0.finalize()
        _CACHE["nc"] = nc0
    nc = _CACHE["nc"]

    in_maps = []
    for c in range(N_CORES):
        hs = slice(c * DPC, (c + 1) * DPC)
        in_maps.append({
            "x": xf, "sxp": sxp, "isx": isx_bc,
            "wqT": np.ascontiguousarray(wq_e[hs, :].T),
            "wkT": np.ascontiguousarray(wk_e[hs, :].T),
            "wvT": np.ascontiguousarray(wv_e[hs, :].T),
            "woT": np.ascontiguousarray(wo_e[hs, :].T).astype(ml_dtypes.bfloat16),
            "cmap": cmap, "smap": smap, "pswapT": pswapT,
            "umask01": um01, "sel2": sel2, "ones1": ones1, "identR": ident,
            "identB": identB,
        })

    res = run_bass_kernel_spmd(nc, in_maps, core_ids=list(range(N_CORES)))
    outp = np.empty((NT, D), np.float32)
    for c in range(N_CORES):
        outp[:, c * DPC:(c + 1) * DPC] = res.results[c]["out"].T
    return outp.reshape(B, T, D)


# revision 37
# speedup vs baseline: 1.0916x; 1.0916x over previous
"""BitNet-style attention layer (B=2, T=2048, D=1024, 16 heads, RoPE, causal)
on 8 TRN2 NeuronCores.

Sharding: head-parallel attention (2 heads/core); wo is computed per-core for
an o-slice after an AllGather of the int8-valued (bf16-stored) quantized
attention output.  Per-token output-quant scales come from a per-batch
AllReduce(max) pipelined behind the attention of the other batch.

Pipeline layout (single fused graph):
  A: per 512-token block: DMA x, quantize (scalar+vector), PE-transpose to
     f32r, QKV projections, in-place RoPE, V-transpose (+ones column for
     fused softmax row-sums).
  B: attention as a flat 2-ahead software-pipelined stream per batch; causal
     masking is a 0/1 multiply on A (vector) so the PE never touches masks;
     the epilogue (amax -> AllReduce -> quantize -> AllGather -> wo) runs
     per batch, hooked into the other batch's stream.
"""

import math
from contextlib import ExitStack

import ml_dtypes
import numpy as np

import concourse.bass as bass
import concourse.bacc as bacc_mod
import concourse.mybir as mybir
import concourse.tile as tile
from concourse.bass_utils import run_bass_kernel_spmd

F32 = mybir.dt.float32
F32R = mybir.dt.float32r
BF16 = mybir.dt.bfloat16
OP = mybir.AluOpType
ACT = mybir.ActivationFunctionType

B, T, D = 2, 2048, 1024
NT = B * T              # 4096 tokens
NH, HD = 16, 64
HDP1 = HD + 1           # V augmented with a ones column (fused row-sum)
N_CORES = 8
HPC = NH // N_CORES     # heads per core = 2
DPC = HPC * HD          # dims per core = 128
RC = 12582912.0         # 1.5*2^23: round-to-nearest-even constant

TB = 512                # token block (matmul N; HW moving-dim max)
NTB = NT // TB          # 8
NTT = NT // 128         # 32 token tiles
QB = 512                # q block
NQB = T // QB           # 4 per batch
NKT = T // 128          # 16 k tiles per batch
TTPB = T // 128         # 16 token tiles per batch


def _quant_w(w):
    O, I = w.shape
    wg = w.reshape(O, I // 128, 128)
    ws = np.abs(wg).mean(-1, keepdims=True) + 1e-5
    wq = np.clip(np.round(wg / ws), -1.0, 1.0) * ws
    return wq.reshape(O, I).astype(np.float32)


def build_nc():
    nc = bacc_mod.Bacc(num_devices=N_CORES)
    io = {}

    def inp(name, shape, dt=F32):
        io[name] = nc.dram_tensor(name, shape, dt, kind="ExternalInput")

    inp("x", [NT, D])
    inp("sxp", [128, NTT])
    inp("isx", [128, NT])
    inp("wqT", [D, DPC], F32R)
    inp("wkT", [D, DPC], F32R)
    inp("wvT", [D, DPC], F32R)
    inp("woT", [D, DPC], BF16)
    inp("cmap", [128, NT])
    inp("smap", [128, NT])
    inp("pswapT", [128, 128], F32R)
    inp("umask01", [128, 4 * 2 * QB], BF16)   # keep-mask, both head halves
    inp("sel2", [33, 128], F32R)
    inp("ones1", [1, 128], F32R)
    inp("identR", [128, 128], F32R)
    inp("identB", [128, 128], BF16)
    out = nc.dram_tensor("out", [DPC, NT], F32, kind="ExternalOutput")

    r32 = lambda ap: ap.bitcast(F32R)
    RG = [list(range(N_CORES))]

    with nc.allow_low_precision(reason="f32r matmul pipeline (FP22 mantissa is ample here)"), \
         tile.TileContext(nc) as tc, ExitStack() as top:
        cpool = top.enter_context(tc.tile_pool(name="const", bufs=1))
        dpool = top.enter_context(tc.tile_pool(name="dram", bufs=1, space="DRAM"))

        # ---- constants (hot ones first on sync; the rest on gpsimd, in
        # approximate order of first use, so phase A starts immediately)
        def const_tile(name, shape, dt=F32, src=None, q=None):
            t = cpool.tile(shape, dt, tag=name)
            (q or nc.gpsimd).dma_start(t[:], src if src is not None
                                       else io[name][:])
            return t

        sxp = const_tile("sxp", [128, NTT], q=nc.sync)
        identB = const_tile("identB", [128, 128], BF16, q=nc.sync)
        wq_sb = [const_tile(f"wq{i}", [128, DPC], F32R, io["wqT"][i * 128:(i + 1) * 128, :]) for i in range(8)]
        wk_sb = [const_tile(f"wk{i}", [128, DPC], F32R, io["wkT"][i * 128:(i + 1) * 128, :]) for i in range(8)]
        wv_sb = [const_tile(f"wv{i}", [128, DPC], F32R, io["wvT"][i * 128:(i + 1) * 128, :]) for i in range(8)]
        pswapT = const_tile("pswapT", [128, 128], F32R)
        identR = const_tile("identR", [128, 128], F32R)
        umask01 = const_tile("umask01", [128, 4 * 2 * QB], BF16)
        sel2 = const_tile("sel2", [33, 128], F32R)
        ones1 = const_tile("ones1", [1, 128], F32R)
        wo_sb = [const_tile(f"wo{i}", [128, DPC], BF16, io["woT"][i * 128:(i + 1) * 128, :]) for i in range(8)]

        # ---- persistent SBUF state
        es_qk = ExitStack()
        qkp = es_qk.enter_context(tc.tile_pool(name="qk", bufs=1))
        qT = qkp.tile([128, NT], F32R, name="qT", tag="qT")
        kT = qkp.tile([128, NT], F32R, name="kT", tag="kT")
        es_va = ExitStack()
        vap = es_va.enter_context(tc.tile_pool(name="vap", bufs=1))
        vaug = [[None] * NTT for _ in range(HPC)]
        es_v = ExitStack()
        vp = es_v.enter_context(tc.tile_pool(name="vp", bufs=1))
        vT = vp.tile([128, NT], F32R, name="vT", tag="vT")

        # ---- per-batch / per-segment DRAM collective buffers
        ar_in = [dpool.tile([128, TTPB], F32, name=f"ar_in{b}", tag=f"ar_in{b}")
                 for b in range(B)]
        ar_out = [dpool.tile([128, TTPB], F32, name=f"ar_out{b}",
                             tag=f"ar_out{b}", addr_space="Shared")
                  for b in range(B)]
        # AG segments: (batch, tok_start, n_tok); b1 split for tail overlap
        AGSEG = [(0, 0, 2048), (1, 2048, 1024), (1, 3072, 1024)]
        ag_in = [dpool.tile([128, n], BF16, name=f"ag_in{s}", tag=f"ag_in{s}")
                 for s, (_, _, n) in enumerate(AGSEG)]
        ag_out = [dpool.tile([N_CORES * 128, n], BF16, name=f"ag_out{s}",
                             tag=f"ag_out{s}", addr_space="Shared")
                  for s, (_, _, n) in enumerate(AGSEG)]

        # ======== Phase A: quantize x, transpose, QKV proj, RoPE, V-transpose
        with tc.tile_pool(name="pxt", bufs=2) as pxt, \
             tc.tile_pool(name="pxf", bufs=2) as pxf, \
             tc.tile_pool(name="pm", bufs=2) as pm, \
             tc.tile_pool(name="ptmp", bufs=2) as ptmp, \
             tc.tile_pool(name="ptp", bufs=2, space="PSUM") as ptp, \
             tc.tile_pool(name="ppp", bufs=2, space="PSUM") as ppp, \
             tc.tile_pool(name="prp", bufs=2, space="PSUM") as prp, \
             tc.tile_pool(name="pvt", bufs=2, space="PSUM") as pvt:
            for tb in range(NTB):
                sl = slice(tb * TB, (tb + 1) * TB)
                xf = [pxf.tile([128, TB], F32R, name=f"xf{i}", tag=f"xf{i}")
                      for i in range(8)]
                for lt in range(4):
                    tt = tb * 4 + lt
                    xt = pxt.tile([128, D], F32, name="xt", tag="xt")
                    nc.sync.dma_start(xt[:], io["x"][tt * 128:(tt + 1) * 128, :])
                    y = pxt.tile([128, D], F32, name="y", tag="y")
                    nc.scalar.activation(y[:], xt[:], ACT.Copy, bias=RC,
                                         scale=sxp[:, tt:tt + 1])
                    xi = pxt.tile([128, D], BF16, name="xi", tag="xi")
                    nc.vector.tensor_scalar(xi[:], y[:], RC, None, OP.subtract)
                    for i in range(8):
                        tp = ptp.tile([128, 128], BF16, name="tp", tag="tp")
                        nc.tensor.transpose(tp[:], xi[:, i * 128:(i + 1) * 128],
                                            identB[:])
                        dst = xf[i][:, lt * 128:(lt + 1) * 128]
                        if i < 4:
                            nc.scalar.copy(dst, tp[:])
                        else:
                            nc.vector.tensor_copy(dst, tp[:])
                # projections
                isxb = pm.tile([128, TB], F32, name="isxb", tag="isxb")
                nc.sync.dma_start(isxb[:], io["isx"][:, sl])
                cm = pm.tile([128, TB], F32, name="cm", tag="cm")
                nc.sync.dma_start(cm[:], io["cmap"][:, sl])
                sm = pm.tile([128, TB], F32, name="sm", tag="sm")
                nc.sync.dma_start(sm[:], io["smap"][:, sl])
                for w_sb, dstT in ((wq_sb, qT), (wk_sb, kT), (wv_sb, vT)):
                    pp = ppp.tile([128, TB], F32, name="pp", tag="pp")
                    for i in range(8):
                        nc.tensor.matmul(pp[:], w_sb[i][:], xf[i][:],
                                         start=(i == 0), stop=(i == 7))
                    nc.vector.tensor_tensor(dstT[:, sl], pp[:], isxb[:],
                                            OP.mult)
                # RoPE in place on q, k
                for srcT in (qT, kT):
                    swp = prp.tile([128, TB], F32, name="swp", tag="swp")
                    nc.tensor.matmul(swp[:], pswapT[:], srcT[:, sl],
                                     start=True, stop=True)
                    tmp = ptmp.tile([128, TB], F32, name="tmp", tag="tmp")
                    nc.gpsimd.tensor_tensor(tmp[:], srcT[:, sl], cm[:], OP.mult)
                    tmp2 = ptmp.tile([128, TB], F32, name="tmp2", tag="tmp2")
                    nc.vector.tensor_tensor(tmp2[:], swp[:], sm[:], OP.mult)
                    nc.gpsimd.tensor_tensor(srcT[:, sl], tmp[:], tmp2[:], OP.add)
                # V transpose (+ones column) for this block's 4 k-tiles
                for lt in range(4):
                    kt = tb * 4 + lt
                    vtp = pvt.tile([128, 128], F32, name="vtp", tag="vtp")
                    nc.tensor.transpose(r32(vtp[:]),
                                        vT[:, kt * 128:(kt + 1) * 128],
                                        identR[:])
                    va0 = vap.tile([128, HDP1], BF16, name=f"va0_{kt}",
                                   tag=f"va0_{kt}")
                    nc.vector.memset(va0[:, HD:HDP1], 1.0)
                    nc.scalar.copy(va0[:, 0:HD], vtp[:, 0:HD])
                    va1 = vap.tile([128, HDP1], BF16, name=f"va1_{kt}",
                                   tag=f"va1_{kt}")
                    nc.vector.memset(va1[:, HD:HDP1], 1.0)
                    nc.vector.tensor_copy(va1[:, 0:HD], vtp[:, HD:128])
                    vaug[0][kt] = va0
                    vaug[1][kt] = va1
        es_v.close()

        # ======== Phase B: attention with per-batch pipelined epilogue
        es_b = ExitStack()
        big = es_b.enter_context(tc.tile_pool(name="big", bufs=1))
        out_n = big.tile([128, NT], F32R, name="out_n", tag="out_n")
        xio = big.tile([128, NT], BF16, name="xio", tag="xio")
        rsi = big.tile([33, QB], F32, name="rsi", tag="rsi")
        rsr = big.tile([33, QB], F32R, name="rsr", tag="rsr")
        nc.vector.memset(rsi[:], 1.0)
        prow = es_b.enter_context(tc.tile_pool(name="prow", bufs=2))
        so_rows = [None] * B
        iso_rows = [None] * B
        psSp = es_b.enter_context(tc.tile_pool(name="psS", bufs=3, space="PSUM"))
        pAcc = es_b.enter_context(tc.tile_pool(name="pAcc", bufs=1, space="PSUM"))
        pA = es_b.enter_context(tc.tile_pool(name="pA", bufs=3))
        pscl = es_b.enter_context(tc.tile_pool(name="pscl", bufs=2))
        pyq = es_b.enter_context(tc.tile_pool(name="pyq", bufs=2))
        pg = es_b.enter_context(tc.tile_pool(name="pg", bufs=2))
        pfin = es_b.enter_context(tc.tile_pool(name="pfin", bufs=2))

        def post_ps():
            # post-work PSUM tiles share the psS pool (PSUM is fully budgeted)
            return psSp.tile([128, 2 * QB], F32, name="psS", tag="psS")

        def attn_batch(b, hooks=None):
            """Whole-batch attention as one flat software-pipelined stream:
            scores run 2 (qb, kl)-stages ahead of A@V, so neither the exp
            latency nor the per-qb epilogue ever starves the PE queue.
            hooks[i] is emitted after pipeline step i (mid-batch interleave
            of the other batch's post-processing)."""
            seq = [(qb, kl) for qb in range(NQB) for kl in range(4 * qb + 4)]
            n = len(seq)
            psA = {}
            psS_l = {}
            A_l = {}

            def scores(i):
                qb, kl = seq[i]
                # diag band tile v: queries < v*128 are fully masked — skip
                off = max(0, kl - 4 * qb) * 128
                qsl = slice(b * T + qb * QB + off, b * T + (qb + 1) * QB)
                kt = b * NKT + kl
                ksl = slice(kt * 128, (kt + 1) * 128)
                psS = psSp.tile([128, 2 * QB], F32, name="psS", tag="psS")
                for h in range(HPC):
                    hsl = slice(h * HD, (h + 1) * HD)
                    ssl = slice(h * QB + off, (h + 1) * QB)
                    nc.tensor.matmul(psS[:, ssl], kT[hsl, ksl], qT[hsl, qsl],
                                     start=True, stop=True)
                psS_l[i] = psS

            def do_exp(i):
                qb, kl = seq[i]
                A = pA.tile([128, 2 * QB], BF16, name="A", tag="A")
                nc.scalar.activation(A[:], psS_l.pop(i)[:], ACT.Exp,
                                     scale=1.0 / math.sqrt(HD))
                if kl >= 4 * qb:     # diagonal band: zero the future keys
                    v = kl - 4 * qb
                    nc.vector.tensor_tensor(
                        A[:], A[:], umask01[:, v * 2 * QB:(v + 1) * 2 * QB],
                        OP.mult)
                A_l[i] = A

            def av(i):
                qb, kl = seq[i]
                nkt = 4 * qb + 4
                if kl == 0:
                    psA[qb] = (pAcc.tile([128, QB], F32, name="psA0", tag="psA0"),
                               pAcc.tile([128, QB], F32, name="psA1", tag="psA1"))
                psA0, psA1 = psA[qb]
                kt = b * NKT + kl
                A = A_l.pop(i)
                off = max(0, kl - 4 * qb) * 128
                st, sp = kl == 0, kl == nkt - 1
                nc.tensor.matmul(psA0[0:HDP1, off:QB], vaug[0][kt][:],
                                 A[:, off:QB], start=st, stop=sp,
                                 skip_group_check=True)
                nc.tensor.matmul(psA1[0:HDP1, off:QB], vaug[1][kt][:],
                                 A[:, QB + off:2 * QB], start=st, stop=sp,
                                 skip_group_check=True)
                if sp:
                    epilogue(qb, psA.pop(qb))

            def epilogue(qb, psA01):
                psA0, psA1 = psA01
                qsl = slice(b * T + qb * QB, b * T + (qb + 1) * QB)
                # fused row-sums live at psA0[HD], psA1[HD]
                nc.vector.tensor_copy(rsi[0:1, :], psA0[HD:HDP1, :])
                nc.vector.tensor_copy(rsi[32:33, :], psA1[HD:HDP1, :])
                nc.vector.reciprocal(rsr[:], rsi[:])
                brs = post_ps()
                nc.tensor.matmul(brs[:, 0:QB], sel2[:], rsr[:], start=True,
                                 stop=True)
                brs_sb = pyq.tile([128, QB], F32, name="brs_sb", tag="brs_sb")
                nc.vector.tensor_copy(brs_sb[:], brs[:, 0:QB])
                nc.vector.tensor_tensor(out_n[0:HD, qsl], psA0[0:HD, :],
                                        brs_sb[0:HD, :], OP.mult)
                nc.vector.tensor_tensor(out_n[HD:128, qsl], psA1[0:HD, :],
                                        brs_sb[HD:128, :], OP.mult)

            scores(0)
            scores(1)
            do_exp(0)
            for i in range(2, n):
                scores(i)
                do_exp(i - 1)
                av(i - 2)
                if hooks and i in hooks:
                    hooks[i]()
            do_exp(n - 1)
            av(n - 2)
            av(n - 1)

        def amax_ar(b):
            am = pscl.tile([128, TTPB], F32, name="am", tag="am")
            for j in range(TTPB):
                gtt = b * TTPB + j
                tp = post_ps()
                nc.tensor.transpose(r32(tp[:, 0:128]),
                                    out_n[:, gtt * 128:(gtt + 1) * 128],
                                    identR[:])
                nc.vector.tensor_reduce(am[:, j:j + 1], tp[:, 0:128],
                                        mybir.AxisListType.X, OP.max,
                                        apply_absolute_value=True)
            nc.sync.dma_start(ar_in[b][:], am[:])
            nc.gpsimd.collective_compute(
                "AllReduce", OP.max, replica_groups=RG,
                ins=[ar_in[b][:].opt()], outs=[ar_out[b][:].opt()])

        def scale_rows(b):
            gmax = pscl.tile([128, TTPB], F32, name="gmax", tag="gmax")
            nc.sync.dma_start(gmax[:], ar_out[b][:])
            iso_p = pscl.tile([128, TTPB], F32R, name="iso_p", tag="iso_p")
            nc.vector.tensor_scalar(iso_p[:], gmax[:], 1e-5, 1.0 / 127.0,
                                    OP.add, OP.mult)
            so_p = pscl.tile([128, TTPB], F32R, name="so_p", tag="so_p")
            nc.vector.reciprocal(so_p[:], iso_p[:])
            tso = post_ps()
            nc.tensor.transpose(r32(tso[0:TTPB, 0:128]), so_p[:], identR[:])
            so_sq = pscl.tile([TTPB, 128], F32R, name="so_sq", tag="so_sq")
            nc.vector.tensor_copy(so_sq[:], tso[0:TTPB, 0:128])
            tiso = post_ps()
            nc.tensor.transpose(r32(tiso[0:TTPB, 0:128]), iso_p[:], identR[:])
            iso_sq = pscl.tile([TTPB, 128], F32R, name="iso_sq", tag="iso_sq")
            nc.vector.tensor_copy(iso_sq[:], tiso[0:TTPB, 0:128])
            so_rows[b] = prow.tile([1, T], F32R, name="so_row", tag="so_row")
            iso_rows[b] = prow.tile([1, T], F32R, name="iso_row", tag="iso_row")
            nc.sync.dma_start(so_rows[b][0:1, :], so_sq[:, :])
            nc.sync.dma_start(iso_rows[b][0:1, :], iso_sq[:, :])

        def quant_ag(seg):
            b, t0, ntok = AGSEG[seg]
            for tl in range(ntok // TB):
                sl = slice(t0 + tl * TB, t0 + (tl + 1) * TB)
                rsl = slice(t0 - b * T + tl * TB, t0 - b * T + (tl + 1) * TB)
                bso = post_ps()
                nc.tensor.matmul(bso[:, 0:QB], ones1[:], so_rows[b][:, rsl],
                                 start=True, stop=True)
                yq = pyq.tile([128, TB], F32, name="yq", tag="yq")
                nc.vector.tensor_tensor(yq[:], out_n[:, sl], bso[:, 0:QB],
                                        OP.mult)
                nc.vector.tensor_scalar(xio[:, sl], yq[:], RC, RC,
                                        OP.add, OP.subtract)
            nc.sync.dma_start(ag_in[seg][:], xio[:, t0:t0 + ntok])
            nc.gpsimd.collective_compute(
                "AllGather", OP.bypass, replica_groups=RG,
                ins=[ag_in[seg][:].opt()], outs=[ag_out[seg][:].opt()])

        def wo(seg):
            b, t0, ntok = AGSEG[seg]
            for tl in range(ntok // TB):
                sl = slice(t0 + tl * TB, t0 + (tl + 1) * TB)
                rsl = slice(t0 - b * T + tl * TB, t0 - b * T + (tl + 1) * TB)
                pw = post_ps()
                for i in range(8):
                    g = pg.tile([128, TB], BF16, name=f"g{i}", tag=f"g{i}")
                    nc.sync.dma_start(g[:], ag_out[seg][i * 128:(i + 1) * 128,
                                                       tl * TB:(tl + 1) * TB])
                    nc.tensor.matmul(pw[:, 0:QB], wo_sb[i][:], g[:],
                                     start=(i == 0), stop=(i == 7))
                bi = post_ps()
                nc.tensor.matmul(bi[:, 0:QB], ones1[:], iso_rows[b][:, rsl],
                                 start=True, stop=True)
                bi_sb = pyq.tile([128, TB], F32, name="bi_sb", tag="bi_sb")
                nc.scalar.copy(bi_sb[:], bi[:, 0:QB])
                fin = pfin.tile([128, TB], F32, name="fin", tag="fin")
                nc.vector.tensor_tensor(fin[:], pw[:, 0:QB], bi_sb[:], OP.mult)
                nc.sync.dma_start(out[:, sl], fin[:])

        def post_b0():
            scale_rows(0)
            quant_ag(0)

        attn_batch(0)
        amax_ar(0)
        attn_batch(1, hooks={18: post_b0})
        amax_ar(1)
        wo(0)
        scale_rows(1)
        quant_ag(1)
        quant_ag(2)
        wo(1)
        wo(2)

        es_b.close()
        es_va.close()
        es_qk.close()

    return nc


_CACHE = {}


def kernel(x, cos, sin, wq_w, wk_w, wv_w, wo_w):
    x = np.asarray(x, np.float32)
    cos = np.asarray(cos, np.float32)   # [T, 32]
    sin = np.asarray(sin, np.float32)
    xf = np.ascontiguousarray(x.reshape(NT, D))

    amax = np.abs(xf).max(-1) + 1e-5
    sx = (127.0 / amax).astype(np.float32)
    isx = (amax / 127.0).astype(np.float32)
    sxp = np.ascontiguousarray(sx.reshape(NTT, 128).T)
    isx_bc = np.ascontiguousarray(np.broadcast_to(isx[None, :], (128, NT)))

    # RoPE maps from the provided cos/sin tables
    cm64 = np.repeat(cos.T, 2, axis=0)            # [64, T]
    sm64 = np.repeat(sin.T, 2, axis=0)
    # rows: [64 dims for head-even][64 dims for head-odd]; cols: [b0 | b1]
    cmap = np.tile(np.concatenate([cm64, cm64], axis=0), (1, B)).astype(np.float32)
    smap = np.tile(np.concatenate([sm64, sm64], axis=0), (1, B)).astype(np.float32)

    P = np.zeros((128, 128), np.float32)
    for j in range(64):
        P[2 * j, 2 * j + 1] = -1.0
        P[2 * j + 1, 2 * j] = 1.0
    pswapT = np.ascontiguousarray(P.T)
    kk = np.arange(128)[:, None]
    qq = np.arange(QB)[None, :]
    # keep-mask (1 = attend, 0 = future), duplicated for both head halves
    um1 = [((v * 128 + kk) <= qq).astype(np.float32) for v in range(4)]
    um01 = np.concatenate([np.concatenate([m, m], axis=1) for m in um1],
                          axis=1).astype(ml_dtypes.bfloat16)
    sel2 = np.zeros((33, 128), np.float32)
    sel2[0, 0:HD] = 1.0
    sel2[32, HD:128] = 1.0
    ones1 = np.ones((1, 128), np.float32)
    ident = np.eye(128, dtype=np.float32)
    identB = np.eye(128, dtype=np.float32).astype(ml_dtypes.bfloat16)

    wq_e, wk_e, wv_e, wo_e = (_quant_w(np.asarray(w, np.float32))
                              for w in (wq_w, wk_w, wv_w, wo_w))

    if "nc" not in _CACHE:
        nc0 = build_nc()
        nc0.finalize()
        _CACHE["nc"] = nc0
    nc = _CACHE["nc"]

    in_maps = []
    for c in range(N_CORES):
        hs = slice(c * DPC, (c + 1) * DPC)
        in_maps.append({
            "x": xf, "sxp": sxp, "isx": isx_bc,
            "wqT": np.ascontiguousarray(wq_e[hs, :].T),
            "wkT": np.ascontiguousarray(wk_e[hs, :].T),
            "wvT": np.ascontiguousarray(wv_e[hs, :].T),
            "woT": np.ascontiguousarray(wo_e[hs, :].T).astype(ml_dtypes.bfloat16),
            "cmap": cmap, "smap": smap, "pswapT": pswapT,
            "umask01": um01, "sel2": sel2, "ones1": ones1, "identR": ident,
            "identB": identB,
        })

    res = run_bass_kernel_spmd(nc, in_maps, core_ids=list(range(N_CORES)))
    outp = np.empty((NT, D), np.float32)
    for c in range(N_CORES):
        outp[:, c * DPC:(c + 1) * DPC] = res.results[c]["out"].T
    return outp.reshape(B, T, D)


# revision 38
# speedup vs baseline: 1.0928x; 1.0011x over previous
"""BitNet-style attention layer (B=2, T=2048, D=1024, 16 heads, RoPE, causal)
on 8 TRN2 NeuronCores.

Sharding: head-parallel attention (2 heads/core); wo is computed per-core for
an o-slice after an AllGather of the int8-valued (bf16-stored) quantized
attention output.  Per-token output-quant scales come from a per-batch
AllReduce(max) pipelined behind the attention of the other batch.

Pipeline layout (single fused graph):
  A: per 512-token block: DMA x, quantize (scalar+vector), PE-transpose to
     f32r, QKV projections, in-place RoPE, V-transpose (+ones column for
     fused softmax row-sums).
  B: attention as a flat 2-ahead software-pipelined stream per batch; causal
     masking is a 0/1 multiply on A (vector) so the PE never touches masks;
     the epilogue (amax -> AllReduce -> quantize -> AllGather -> wo) runs
     per batch, hooked into the other batch's stream.
"""

import math
from contextlib import ExitStack

import ml_dtypes
import numpy as np

import concourse.bass as bass
import concourse.bacc as bacc_mod
import concourse.mybir as mybir
import concourse.tile as tile
from concourse.bass_utils import run_bass_kernel_spmd

F32 = mybir.dt.float32
F32R = mybir.dt.float32r
BF16 = mybir.dt.bfloat16
OP = mybir.AluOpType
ACT = mybir.ActivationFunctionType

B, T, D = 2, 2048, 1024
NT = B * T              # 4096 tokens
NH, HD = 16, 64
HDP1 = HD + 1           # V augmented with a ones column (fused row-sum)
N_CORES = 8
HPC = NH // N_CORES     # heads per core = 2
DPC = HPC * HD          # dims per core = 128
RC = 12582912.0         # 1.5*2^23: round-to-nearest-even constant

TB = 512                # token block (matmul N; HW moving-dim max)
NTB = NT // TB          # 8
NTT = NT // 128         # 32 token tiles
QB = 512                # q block
NQB = T // QB           # 4 per batch
NKT = T // 128          # 16 k tiles per batch
TTPB = T // 128         # 16 token tiles per batch


def _quant_w(w):
    O, I = w.shape
    wg = w.reshape(O, I // 128, 128)
    ws = np.abs(wg).mean(-1, keepdims=True) + 1e-5
    wq = np.clip(np.round(wg / ws), -1.0, 1.0) * ws
    return wq.reshape(O, I).astype(np.float32)


def build_nc():
    nc = bacc_mod.Bacc(num_devices=N_CORES)
    io = {}

    def inp(name, shape, dt=F32):
        io[name] = nc.dram_tensor(name, shape, dt, kind="ExternalInput")

    inp("x", [NT, D])
    inp("sxp", [128, NTT])
    inp("isx", [128, NT])
    inp("wqT", [D, DPC], F32R)
    inp("wkT", [D, DPC], F32R)
    inp("wvT", [D, DPC], F32R)
    inp("woT", [D, DPC], BF16)
    inp("cmap", [128, NT])
    inp("smap", [128, NT])
    inp("pswapT", [128, 128], F32R)
    inp("umask01", [128, 4 * 2 * QB], BF16)   # keep-mask, both head halves
    inp("sel2", [33, 128], F32R)
    inp("ones1", [1, 128], F32R)
    inp("identR", [128, 128], F32R)
    inp("identB", [128, 128], BF16)
    out = nc.dram_tensor("out", [DPC, NT], F32, kind="ExternalOutput")

    r32 = lambda ap: ap.bitcast(F32R)
    RG = [list(range(N_CORES))]

    with nc.allow_low_precision(reason="f32r matmul pipeline (FP22 mantissa is ample here)"), \
         tile.TileContext(nc) as tc, ExitStack() as top:
        cpool = top.enter_context(tc.tile_pool(name="const", bufs=1))
        dpool = top.enter_context(tc.tile_pool(name="dram", bufs=1, space="DRAM"))

        # ---- constants (hot ones first on sync; the rest on gpsimd, in
        # approximate order of first use, so phase A starts immediately)
        def const_tile(name, shape, dt=F32, src=None, q=None):
            t = cpool.tile(shape, dt, tag=name)
            (q or nc.gpsimd).dma_start(t[:], src if src is not None
                                       else io[name][:])
            return t

        sxp = const_tile("sxp", [128, NTT], q=nc.sync)
        identB = const_tile("identB", [128, 128], BF16, q=nc.sync)
        wq_sb = [const_tile(f"wq{i}", [128, DPC], F32R, io["wqT"][i * 128:(i + 1) * 128, :]) for i in range(8)]
        wk_sb = [const_tile(f"wk{i}", [128, DPC], F32R, io["wkT"][i * 128:(i + 1) * 128, :]) for i in range(8)]
        wv_sb = [const_tile(f"wv{i}", [128, DPC], F32R, io["wvT"][i * 128:(i + 1) * 128, :]) for i in range(8)]
        pswapT = const_tile("pswapT", [128, 128], F32R)
        identR = const_tile("identR", [128, 128], F32R)
        umask01 = const_tile("umask01", [128, 4 * 2 * QB], BF16)
        sel2 = const_tile("sel2", [33, 128], F32R)
        ones1 = const_tile("ones1", [1, 128], F32R)
        wo_sb = [const_tile(f"wo{i}", [128, DPC], BF16, io["woT"][i * 128:(i + 1) * 128, :]) for i in range(8)]

        # ---- persistent SBUF state
        es_qk = ExitStack()
        qkp = es_qk.enter_context(tc.tile_pool(name="qk", bufs=1))
        qT = qkp.tile([128, NT], F32R, name="qT", tag="qT")
        kT = qkp.tile([128, NT], F32R, name="kT", tag="kT")
        es_va = ExitStack()
        vap = es_va.enter_context(tc.tile_pool(name="vap", bufs=1))
        vaug = [[None] * NTT for _ in range(HPC)]
        es_v = ExitStack()
        vp = es_v.enter_context(tc.tile_pool(name="vp", bufs=1))
        vT = vp.tile([128, NT], F32R, name="vT", tag="vT")

        # ---- per-batch / per-segment DRAM collective buffers
        ar_in = [dpool.tile([128, TTPB], F32, name=f"ar_in{b}", tag=f"ar_in{b}")
                 for b in range(B)]
        ar_out = [dpool.tile([128, TTPB], F32, name=f"ar_out{b}",
                             tag=f"ar_out{b}", addr_space="Shared")
                  for b in range(B)]
        # AG segments: (batch, tok_start, n_tok); b1 split for tail overlap
        AGSEG = [(0, 0, 2048), (1, 2048, 1024), (1, 3072, 1024)]
        ag_in = [dpool.tile([128, n], BF16, name=f"ag_in{s}", tag=f"ag_in{s}")
                 for s, (_, _, n) in enumerate(AGSEG)]
        ag_out = [dpool.tile([N_CORES * 128, n], BF16, name=f"ag_out{s}",
                             tag=f"ag_out{s}", addr_space="Shared")
                  for s, (_, _, n) in enumerate(AGSEG)]

        # ======== Phase A: quantize x, transpose, QKV proj, RoPE, V-transpose
        with tc.tile_pool(name="pxt", bufs=3) as pxt, \
             tc.tile_pool(name="pxf", bufs=2) as pxf, \
             tc.tile_pool(name="pm", bufs=2) as pm, \
             tc.tile_pool(name="ptmp", bufs=2) as ptmp, \
             tc.tile_pool(name="ptp", bufs=3, space="PSUM") as ptp, \
             tc.tile_pool(name="ppp", bufs=2, space="PSUM") as ppp, \
             tc.tile_pool(name="prp", bufs=2, space="PSUM") as prp, \
             tc.tile_pool(name="pvt", bufs=1, space="PSUM") as pvt:
            for tb in range(NTB):
                sl = slice(tb * TB, (tb + 1) * TB)
                xf = [pxf.tile([128, TB], F32R, name=f"xf{i}", tag=f"xf{i}")
                      for i in range(8)]
                for lt in range(4):
                    tt = tb * 4 + lt
                    xt = pxt.tile([128, D], F32, name="xt", tag="xt")
                    nc.sync.dma_start(xt[:], io["x"][tt * 128:(tt + 1) * 128, :])
                    y = pxt.tile([128, D], F32, name="y", tag="y")
                    nc.scalar.activation(y[:], xt[:], ACT.Copy, bias=RC,
                                         scale=sxp[:, tt:tt + 1])
                    xi = pxt.tile([128, D], BF16, name="xi", tag="xi")
                    nc.vector.tensor_scalar(xi[:], y[:], RC, None, OP.subtract)
                    for i in range(8):
                        tp = ptp.tile([128, 128], BF16, name="tp", tag="tp")
                        nc.tensor.transpose(tp[:], xi[:, i * 128:(i + 1) * 128],
                                            identB[:])
                        dst = xf[i][:, lt * 128:(lt + 1) * 128]
                        if i < 4:
                            nc.scalar.copy(dst, tp[:])
                        else:
                            nc.vector.tensor_copy(dst, tp[:])
                # projections
                isxb = pm.tile([128, TB], F32, name="isxb", tag="isxb")
                nc.sync.dma_start(isxb[:], io["isx"][:, sl])
                cm = pm.tile([128, TB], F32, name="cm", tag="cm")
                nc.sync.dma_start(cm[:], io["cmap"][:, sl])
                sm = pm.tile([128, TB], F32, name="sm", tag="sm")
                nc.sync.dma_start(sm[:], io["smap"][:, sl])
                for w_sb, dstT in ((wq_sb, qT), (wk_sb, kT), (wv_sb, vT)):
                    pp = ppp.tile([128, TB], F32, name="pp", tag="pp")
                    for i in range(8):
                        nc.tensor.matmul(pp[:], w_sb[i][:], xf[i][:],
                                         start=(i == 0), stop=(i == 7))
                    nc.vector.tensor_tensor(dstT[:, sl], pp[:], isxb[:],
                                            OP.mult)
                # RoPE in place on q, k
                for srcT in (qT, kT):
                    swp = prp.tile([128, TB], F32, name="swp", tag="swp")
                    nc.tensor.matmul(swp[:], pswapT[:], srcT[:, sl],
                                     start=True, stop=True)
                    tmp = ptmp.tile([128, TB], F32, name="tmp", tag="tmp")
                    nc.gpsimd.tensor_tensor(tmp[:], srcT[:, sl], cm[:], OP.mult)
                    tmp2 = ptmp.tile([128, TB], F32, name="tmp2", tag="tmp2")
                    nc.vector.tensor_tensor(tmp2[:], swp[:], sm[:], OP.mult)
                    nc.gpsimd.tensor_tensor(srcT[:, sl], tmp[:], tmp2[:], OP.add)
                # V transpose (+ones column) for this block's 4 k-tiles
                for lt in range(4):
                    kt = tb * 4 + lt
                    vtp = pvt.tile([128, 128], F32, name="vtp", tag="vtp")
                    nc.tensor.transpose(r32(vtp[:]),
                                        vT[:, kt * 128:(kt + 1) * 128],
                                        identR[:])
                    va0 = vap.tile([128, HDP1], BF16, name=f"va0_{kt}",
                                   tag=f"va0_{kt}")
                    nc.vector.memset(va0[:, HD:HDP1], 1.0)
                    nc.scalar.copy(va0[:, 0:HD], vtp[:, 0:HD])
                    va1 = vap.tile([128, HDP1], BF16, name=f"va1_{kt}",
                                   tag=f"va1_{kt}")
                    nc.vector.memset(va1[:, HD:HDP1], 1.0)
                    nc.vector.tensor_copy(va1[:, 0:HD], vtp[:, HD:128])
                    vaug[0][kt] = va0
                    vaug[1][kt] = va1
        es_v.close()

        # ======== Phase B: attention with per-batch pipelined epilogue
        es_b = ExitStack()
        big = es_b.enter_context(tc.tile_pool(name="big", bufs=1))
        out_n = big.tile([128, NT], F32R, name="out_n", tag="out_n")
        xio = big.tile([128, NT], BF16, name="xio", tag="xio")
        rsi = big.tile([33, QB], F32, name="rsi", tag="rsi")
        rsr = big.tile([33, QB], F32R, name="rsr", tag="rsr")
        nc.vector.memset(rsi[:], 1.0)
        prow = es_b.enter_context(tc.tile_pool(name="prow", bufs=2))
        so_rows = [None] * B
        iso_rows = [None] * B
        psSp = es_b.enter_context(tc.tile_pool(name="psS", bufs=3, space="PSUM"))
        pAcc = es_b.enter_context(tc.tile_pool(name="pAcc", bufs=1, space="PSUM"))
        pA = es_b.enter_context(tc.tile_pool(name="pA", bufs=3))
        pscl = es_b.enter_context(tc.tile_pool(name="pscl", bufs=2))
        pyq = es_b.enter_context(tc.tile_pool(name="pyq", bufs=2))
        pg = es_b.enter_context(tc.tile_pool(name="pg", bufs=2))
        pfin = es_b.enter_context(tc.tile_pool(name="pfin", bufs=2))

        def post_ps():
            # post-work PSUM tiles share the psS pool (PSUM is fully budgeted)
            return psSp.tile([128, 2 * QB], F32, name="psS", tag="psS")

        def attn_batch(b, hooks=None):
            """Whole-batch attention as one flat software-pipelined stream:
            scores run 2 (qb, kl)-stages ahead of A@V, so neither the exp
            latency nor the per-qb epilogue ever starves the PE queue.
            hooks[i] is emitted after pipeline step i (mid-batch interleave
            of the other batch's post-processing)."""
            seq = [(qb, kl) for qb in range(NQB) for kl in range(4 * qb + 4)]
            n = len(seq)
            psA = {}
            psS_l = {}
            A_l = {}

            def scores(i):
                qb, kl = seq[i]
                # diag band tile v: queries < v*128 are fully masked — skip
                off = max(0, kl - 4 * qb) * 128
                qsl = slice(b * T + qb * QB + off, b * T + (qb + 1) * QB)
                kt = b * NKT + kl
                ksl = slice(kt * 128, (kt + 1) * 128)
                psS = psSp.tile([128, 2 * QB], F32, name="psS", tag="psS")
                for h in range(HPC):
                    hsl = slice(h * HD, (h + 1) * HD)
                    ssl = slice(h * QB + off, (h + 1) * QB)
                    nc.tensor.matmul(psS[:, ssl], kT[hsl, ksl], qT[hsl, qsl],
                                     start=True, stop=True)
                psS_l[i] = psS

            def do_exp(i):
                qb, kl = seq[i]
                A = pA.tile([128, 2 * QB], BF16, name="A", tag="A")
                nc.scalar.activation(A[:], psS_l.pop(i)[:], ACT.Exp,
                                     scale=1.0 / math.sqrt(HD))
                if kl >= 4 * qb:     # diagonal band: zero the future keys
                    v = kl - 4 * qb
                    nc.vector.tensor_tensor(
                        A[:], A[:], umask01[:, v * 2 * QB:(v + 1) * 2 * QB],
                        OP.mult)
                A_l[i] = A

            def av(i):
                qb, kl = seq[i]
                nkt = 4 * qb + 4
                if kl == 0:
                    psA[qb] = (pAcc.tile([128, QB], F32, name="psA0", tag="psA0"),
                               pAcc.tile([128, QB], F32, name="psA1", tag="psA1"))
                psA0, psA1 = psA[qb]
                kt = b * NKT + kl
                A = A_l.pop(i)
                off = max(0, kl - 4 * qb) * 128
                st, sp = kl == 0, kl == nkt - 1
                nc.tensor.matmul(psA0[0:HDP1, off:QB], vaug[0][kt][:],
                                 A[:, off:QB], start=st, stop=sp,
                                 skip_group_check=True)
                nc.tensor.matmul(psA1[0:HDP1, off:QB], vaug[1][kt][:],
                                 A[:, QB + off:2 * QB], start=st, stop=sp,
                                 skip_group_check=True)
                if sp:
                    epilogue(qb, psA.pop(qb))

            def epilogue(qb, psA01):
                psA0, psA1 = psA01
                qsl = slice(b * T + qb * QB, b * T + (qb + 1) * QB)
                # fused row-sums live at psA0[HD], psA1[HD]
                nc.vector.tensor_copy(rsi[0:1, :], psA0[HD:HDP1, :])
                nc.vector.tensor_copy(rsi[32:33, :], psA1[HD:HDP1, :])
                nc.vector.reciprocal(rsr[:], rsi[:])
                brs = post_ps()
                nc.tensor.matmul(brs[:, 0:QB], sel2[:], rsr[:], start=True,
                                 stop=True)
                brs_sb = pyq.tile([128, QB], F32, name="brs_sb", tag="brs_sb")
                nc.vector.tensor_copy(brs_sb[:], brs[:, 0:QB])
                nc.vector.tensor_tensor(out_n[0:HD, qsl], psA0[0:HD, :],
                                        brs_sb[0:HD, :], OP.mult)
                nc.vector.tensor_tensor(out_n[HD:128, qsl], psA1[0:HD, :],
                                        brs_sb[HD:128, :], OP.mult)

            scores(0)
            scores(1)
            do_exp(0)
            for i in range(2, n):
                scores(i)
                do_exp(i - 1)
                av(i - 2)
                if hooks and i in hooks:
                    hooks[i]()
            do_exp(n - 1)
            av(n - 2)
            av(n - 1)

        def amax_ar(b):
            am = pscl.tile([128, TTPB], F32, name="am", tag="am")
            for j in range(TTPB):
                gtt = b * TTPB + j
                tp = post_ps()
                nc.tensor.transpose(r32(tp[:, 0:128]),
                                    out_n[:, gtt * 128:(gtt + 1) * 128],
                                    identR[:])
                nc.vector.tensor_reduce(am[:, j:j + 1], tp[:, 0:128],
                                        mybir.AxisListType.X, OP.max,
                                        apply_absolute_value=True)
            nc.sync.dma_start(ar_in[b][:], am[:])
            nc.gpsimd.collective_compute(
                "AllReduce", OP.max, replica_groups=RG,
                ins=[ar_in[b][:].opt()], outs=[ar_out[b][:].opt()])

        def scale_rows(b):
            gmax = pscl.tile([128, TTPB], F32, name="gmax", tag="gmax")
            nc.sync.dma_start(gmax[:], ar_out[b][:])
            iso_p = pscl.tile([128, TTPB], F32R, name="iso_p", tag="iso_p")
            nc.vector.tensor_scalar(iso_p[:], gmax[:], 1e-5, 1.0 / 127.0,
                                    OP.add, OP.mult)
            so_p = pscl.tile([128, TTPB], F32R, name="so_p", tag="so_p")
            nc.vector.reciprocal(so_p[:], iso_p[:])
            tso = post_ps()
            nc.tensor.transpose(r32(tso[0:TTPB, 0:128]), so_p[:], identR[:])
            so_sq = pscl.tile([TTPB, 128], F32R, name="so_sq", tag="so_sq")
            nc.vector.tensor_copy(so_sq[:], tso[0:TTPB, 0:128])
            tiso = post_ps()
            nc.tensor.transpose(r32(tiso[0:TTPB, 0:128]), iso_p[:], identR[:])
            iso_sq = pscl.tile([TTPB, 128], F32R, name="iso_sq", tag="iso_sq")
            nc.vector.tensor_copy(iso_sq[:], tiso[0:TTPB, 0:128])
            so_rows[b] = prow.tile([1, T], F32R, name="so_row", tag="so_row")
            iso_rows[b] = prow.tile([1, T], F32R, name="iso_row", tag="iso_row")
            nc.sync.dma_start(so_rows[b][0:1, :], so_sq[:, :])
            nc.sync.dma_start(iso_rows[b][0:1, :], iso_sq[:, :])

        def quant_ag(seg):
            b, t0, ntok = AGSEG[seg]
            for tl in range(ntok // TB):
                sl = slice(t0 + tl * TB, t0 + (tl + 1) * TB)
                rsl = slice(t0 - b * T + tl * TB, t0 - b * T + (tl + 1) * TB)
                bso = post_ps()
                nc.tensor.matmul(bso[:, 0:QB], ones1[:], so_rows[b][:, rsl],
                                 start=True, stop=True)
                yq = pyq.tile([128, TB], F32, name="yq", tag="yq")
                nc.vector.tensor_tensor(yq[:], out_n[:, sl], bso[:, 0:QB],
                                        OP.mult)
                nc.vector.tensor_scalar(xio[:, sl], yq[:], RC, RC,
                                        OP.add, OP.subtract)
            nc.sync.dma_start(ag_in[seg][:], xio[:, t0:t0 + ntok])
            nc.gpsimd.collective_compute(
                "AllGather", OP.bypass, replica_groups=RG,
                ins=[ag_in[seg][:].opt()], outs=[ag_out[seg][:].opt()])

        def wo(seg):
            b, t0, ntok = AGSEG[seg]
            for tl in range(ntok // TB):
                sl = slice(t0 + tl * TB, t0 + (tl + 1) * TB)
                rsl = slice(t0 - b * T + tl * TB, t0 - b * T + (tl + 1) * TB)
                pw = post_ps()
                for i in range(8):
                    g = pg.tile([128, TB], BF16, name=f"g{i}", tag=f"g{i}")
                    nc.sync.dma_start(g[:], ag_out[seg][i * 128:(i + 1) * 128,
                                                       tl * TB:(tl + 1) * TB])
                    nc.tensor.matmul(pw[:, 0:QB], wo_sb[i][:], g[:],
                                     start=(i == 0), stop=(i == 7))
                bi = post_ps()
                nc.tensor.matmul(bi[:, 0:QB], ones1[:], iso_rows[b][:, rsl],
                                 start=True, stop=True)
                bi_sb = pyq.tile([128, TB], F32, name="bi_sb", tag="bi_sb")
                nc.scalar.copy(bi_sb[:], bi[:, 0:QB])
                fin = pfin.tile([128, TB], F32, name="fin", tag="fin")
                nc.vector.tensor_tensor(fin[:], pw[:, 0:QB], bi_sb[:], OP.mult)
                nc.sync.dma_start(out[:, sl], fin[:])

        def post_b0():
            scale_rows(0)
            quant_ag(0)

        attn_batch(0)
        amax_ar(0)
        attn_batch(1, hooks={18: post_b0})
        amax_ar(1)
        wo(0)
        scale_rows(1)
        quant_ag(1)
        quant_ag(2)
        wo(1)
        wo(2)

        es_b.close()
        es_va.close()
        es_qk.close()

    return nc


_CACHE = {}


def kernel(x, cos, sin, wq_w, wk_w, wv_w, wo_w):
    x = np.asarray(x, np.float32)
    cos = np.asarray(cos, np.float32)   # [T, 32]
    sin = np.asarray(sin, np.float32)
    xf = np.ascontiguousarray(x.reshape(NT, D))

    amax = np.abs(xf).max(-1) + 1e-5
    sx = (127.0 / amax).astype(np.float32)
    isx = (amax / 127.0).astype(np.float32)
    sxp = np.ascontiguousarray(sx.reshape(NTT, 128).T)
    isx_bc = np.ascontiguousarray(np.broadcast_to(isx[None, :], (128, NT)))

    # RoPE maps from the provided cos/sin tables
    cm64 = np.repeat(cos.T, 2, axis=0)            # [64, T]
    sm64 = np.repeat(sin.T, 2, axis=0)
    # rows: [64 dims for head-even][64 dims for head-odd]; cols: [b0 | b1]
    cmap = np.tile(np.concatenate([cm64, cm64], axis=0), (1, B)).astype(np.float32)
    smap = np.tile(np.concatenate([sm64, sm64], axis=0), (1, B)).astype(np.float32)

    P = np.zeros((128, 128), np.float32)
    for j in range(64):
        P[2 * j, 2 * j + 1] = -1.0
        P[2 * j + 1, 2 * j] = 1.0
    pswapT = np.ascontiguousarray(P.T)
    kk = np.arange(128)[:, None]
    qq = np.arange(QB)[None, :]
    # keep-mask (1 = attend, 0 = future), duplicated for both head halves
    um1 = [((v * 128 + kk) <= qq).astype(np.float32) for v in range(4)]
    um01 = np.concatenate([np.concatenate([m, m], axis=1) for m in um1],
                          axis=1).astype(ml_dtypes.bfloat16)
    sel2 = np.zeros((33, 128), np.float32)
    sel2[0, 0:HD] = 1.0
    sel2[32, HD:128] = 1.0
    ones1 = np.ones((1, 128), np.float32)
    ident = np.eye(128, dtype=np.float32)
    identB = np.eye(128, dtype=np.float32).astype(ml_dtypes.bfloat16)

    wq_e, wk_e, wv_e, wo_e = (_quant_w(np.asarray(w, np.float32))
                              for w in (wq_w, wk_w, wv_w, wo_w))

    if "nc" not in _CACHE:
        nc0 = build_nc()
        nc0.finalize()
        _CACHE["nc"] = nc0
    nc = _CACHE["nc"]

    in_maps = []
    for c in range(N_CORES):
        hs = slice(c * DPC, (c + 1) * DPC)
        in_maps.append({
            "x": xf, "sxp": sxp, "isx": isx_bc,
            "wqT": np.ascontiguousarray(wq_e[hs, :].T),
            "wkT": np.ascontiguousarray(wk_e[hs, :].T),
            "wvT": np.ascontiguousarray(wv_e[hs, :].T),
            "woT": np.ascontiguousarray(wo_e[hs, :].T).astype(ml_dtypes.bfloat16),
            "cmap": cmap, "smap": smap, "pswapT": pswapT,
            "umask01": um01, "sel2": sel2, "ones1": ones1, "identR": ident,
            "identB": identB,
        })

    res = run_bass_kernel_spmd(nc, in_maps, core_ids=list(range(N_CORES)))
    outp = np.empty((NT, D), np.float32)
    for c in range(N_CORES):
        outp[:, c * DPC:(c + 1) * DPC] = res.results[c]["out"].T
    return outp.reshape(B, T, D)


# revision 39
# speedup vs baseline: 1.0992x; 1.0058x over previous
"""BitNet-style attention layer (B=2, T=2048, D=1024, 16 heads, RoPE, causal)
on 8 TRN2 NeuronCores.

Sharding: head-parallel attention (2 heads/core); wo is computed per-core for
an o-slice after an AllGather of the int8-valued (bf16-stored) quantized
attention output.  Per-token output-quant scales come from a per-batch
AllReduce(max) pipelined behind the attention of the other batch.

Pipeline layout (single fused graph):
  A: per 512-token block: DMA x, quantize (scalar+vector), PE-transpose to
     f32r, QKV projections, in-place RoPE, V-transpose (+ones column for
     fused softmax row-sums).
  B: attention as a flat 2-ahead software-pipelined stream per batch; causal
     masking is a 0/1 multiply on A (vector) so the PE never touches masks;
     the epilogue (amax -> AllReduce -> quantize -> AllGather -> wo) runs
     per batch, hooked into the other batch's stream.
"""

import math
from contextlib import ExitStack

import ml_dtypes
import numpy as np

import concourse.bass as bass
import concourse.bacc as bacc_mod
import concourse.mybir as mybir
import concourse.tile as tile
from concourse.bass_utils import run_bass_kernel_spmd

F32 = mybir.dt.float32
F32R = mybir.dt.float32r
BF16 = mybir.dt.bfloat16
OP = mybir.AluOpType
ACT = mybir.ActivationFunctionType

B, T, D = 2, 2048, 1024
NT = B * T              # 4096 tokens
NH, HD = 16, 64
HDP1 = HD + 1           # V augmented with a ones column (fused row-sum)
N_CORES = 8
HPC = NH // N_CORES     # heads per core = 2
DPC = HPC * HD          # dims per core = 128
RC = 12582912.0         # 1.5*2^23: round-to-nearest-even constant

TB = 512                # token block (matmul N; HW moving-dim max)
NTB = NT // TB          # 8
NTT = NT // 128         # 32 token tiles
QB = 512                # q block
NQB = T // QB           # 4 per batch
NKT = T // 128          # 16 k tiles per batch
TTPB = T // 128         # 16 token tiles per batch


def _quant_w(w):
    O, I = w.shape
    wg = w.reshape(O, I // 128, 128)
    ws = np.abs(wg).mean(-1, keepdims=True) + 1e-5
    wq = np.clip(np.round(wg / ws), -1.0, 1.0) * ws
    return wq.reshape(O, I).astype(np.float32)


def build_nc():
    nc = bacc_mod.Bacc(num_devices=N_CORES)
    io = {}

    def inp(name, shape, dt=F32):
        io[name] = nc.dram_tensor(name, shape, dt, kind="ExternalInput")

    inp("x", [NT, D])
    inp("sxp", [128, NTT])
    inp("isx", [128, NT])
    inp("wqT", [D, DPC], F32R)
    inp("wkT", [D, DPC], F32R)
    inp("wvT", [D, DPC], F32R)
    inp("woT", [D, DPC], BF16)
    inp("cmap", [128, NT])
    inp("smap", [128, NT])
    inp("pswapT", [128, 128], F32R)
    inp("umask01", [128, 4 * 2 * QB], BF16)   # keep-mask, both head halves
    inp("sel2", [33, 128], F32R)
    inp("ones1", [1, 128], F32R)
    inp("identR", [128, 128], F32R)
    inp("identB", [128, 128], BF16)
    out = nc.dram_tensor("out", [DPC, NT], F32, kind="ExternalOutput")

    r32 = lambda ap: ap.bitcast(F32R)
    RG = [list(range(N_CORES))]

    with nc.allow_low_precision(reason="f32r matmul pipeline (FP22 mantissa is ample here)"), \
         tile.TileContext(nc) as tc, ExitStack() as top:
        cpool = top.enter_context(tc.tile_pool(name="const", bufs=1))
        dpool = top.enter_context(tc.tile_pool(name="dram", bufs=1, space="DRAM"))

        # ---- constants (hot ones first on sync; the rest on gpsimd, in
        # approximate order of first use, so phase A starts immediately)
        def const_tile(name, shape, dt=F32, src=None, q=None):
            t = cpool.tile(shape, dt, tag=name)
            (q or nc.gpsimd).dma_start(t[:], src if src is not None
                                       else io[name][:])
            return t

        sxp = const_tile("sxp", [128, NTT], q=nc.sync)
        identB = const_tile("identB", [128, 128], BF16, q=nc.sync)
        wq_sb = [const_tile(f"wq{i}", [128, DPC], F32R, io["wqT"][i * 128:(i + 1) * 128, :]) for i in range(8)]
        wk_sb = [const_tile(f"wk{i}", [128, DPC], F32R, io["wkT"][i * 128:(i + 1) * 128, :]) for i in range(8)]
        wv_sb = [const_tile(f"wv{i}", [128, DPC], F32R, io["wvT"][i * 128:(i + 1) * 128, :]) for i in range(8)]
        pswapT = const_tile("pswapT", [128, 128], F32R)
        identR = const_tile("identR", [128, 128], F32R)
        umask01 = const_tile("umask01", [128, 4 * 2 * QB], BF16)
        sel2 = const_tile("sel2", [33, 128], F32R)
        ones1 = const_tile("ones1", [1, 128], F32R)
        wo_sb = [const_tile(f"wo{i}", [128, DPC], BF16, io["woT"][i * 128:(i + 1) * 128, :]) for i in range(8)]

        # ---- persistent SBUF state
        es_qk = ExitStack()
        qkp = es_qk.enter_context(tc.tile_pool(name="qk", bufs=1))
        qT = qkp.tile([128, NT], F32R, name="qT", tag="qT")
        kT = qkp.tile([128, NT], F32R, name="kT", tag="kT")
        es_va = ExitStack()
        vap = es_va.enter_context(tc.tile_pool(name="vap", bufs=1))
        vaug = [[None] * NTT for _ in range(HPC)]
        es_v = ExitStack()
        vp = es_v.enter_context(tc.tile_pool(name="vp", bufs=1))
        vT = vp.tile([128, NT], F32R, name="vT", tag="vT")

        # ---- per-batch / per-segment DRAM collective buffers
        ar_in = [dpool.tile([128, TTPB], F32, name=f"ar_in{b}", tag=f"ar_in{b}")
                 for b in range(B)]
        ar_out = [dpool.tile([128, TTPB], F32, name=f"ar_out{b}",
                             tag=f"ar_out{b}", addr_space="Shared")
                  for b in range(B)]
        # AG segments: (batch, tok_start, n_tok); b1 split for tail overlap
        AGSEG = [(0, 0, 2048), (1, 2048, 1536), (1, 3584, 512)]
        ag_in = [dpool.tile([128, n], BF16, name=f"ag_in{s}", tag=f"ag_in{s}")
                 for s, (_, _, n) in enumerate(AGSEG)]
        ag_out = [dpool.tile([N_CORES * 128, n], BF16, name=f"ag_out{s}",
                             tag=f"ag_out{s}", addr_space="Shared")
                  for s, (_, _, n) in enumerate(AGSEG)]

        # ======== Phase A: quantize x, transpose, QKV proj, RoPE, V-transpose
        with tc.tile_pool(name="pxt", bufs=3) as pxt, \
             tc.tile_pool(name="pxf", bufs=2) as pxf, \
             tc.tile_pool(name="pm", bufs=2) as pm, \
             tc.tile_pool(name="ptmp", bufs=2) as ptmp, \
             tc.tile_pool(name="ptp", bufs=3, space="PSUM") as ptp, \
             tc.tile_pool(name="ppp", bufs=2, space="PSUM") as ppp, \
             tc.tile_pool(name="prp", bufs=2, space="PSUM") as prp, \
             tc.tile_pool(name="pvt", bufs=1, space="PSUM") as pvt:
            for tb in range(NTB):
                sl = slice(tb * TB, (tb + 1) * TB)
                xf = [pxf.tile([128, TB], F32R, name=f"xf{i}", tag=f"xf{i}")
                      for i in range(8)]
                for lt in range(4):
                    tt = tb * 4 + lt
                    xt = pxt.tile([128, D], F32, name="xt", tag="xt")
                    nc.sync.dma_start(xt[:], io["x"][tt * 128:(tt + 1) * 128, :])
                    y = pxt.tile([128, D], F32, name="y", tag="y")
                    nc.scalar.activation(y[:], xt[:], ACT.Copy, bias=RC,
                                         scale=sxp[:, tt:tt + 1])
                    xi = pxt.tile([128, D], BF16, name="xi", tag="xi")
                    nc.vector.tensor_scalar(xi[:], y[:], RC, None, OP.subtract)
                    for i in range(8):
                        tp = ptp.tile([128, 128], BF16, name="tp", tag="tp")
                        nc.tensor.transpose(tp[:], xi[:, i * 128:(i + 1) * 128],
                                            identB[:])
                        dst = xf[i][:, lt * 128:(lt + 1) * 128]
                        if i < 4:
                            nc.scalar.copy(dst, tp[:])
                        else:
                            nc.vector.tensor_copy(dst, tp[:])
                # projections
                isxb = pm.tile([128, TB], F32, name="isxb", tag="isxb")
                nc.sync.dma_start(isxb[:], io["isx"][:, sl])
                cm = pm.tile([128, TB], F32, name="cm", tag="cm")
                nc.sync.dma_start(cm[:], io["cmap"][:, sl])
                sm = pm.tile([128, TB], F32, name="sm", tag="sm")
                nc.sync.dma_start(sm[:], io["smap"][:, sl])
                for w_sb, dstT in ((wq_sb, qT), (wk_sb, kT), (wv_sb, vT)):
                    pp = ppp.tile([128, TB], F32, name="pp", tag="pp")
                    for i in range(8):
                        nc.tensor.matmul(pp[:], w_sb[i][:], xf[i][:],
                                         start=(i == 0), stop=(i == 7))
                    nc.vector.tensor_tensor(dstT[:, sl], pp[:], isxb[:],
                                            OP.mult)
                # RoPE in place on q, k
                for srcT in (qT, kT):
                    swp = prp.tile([128, TB], F32, name="swp", tag="swp")
                    nc.tensor.matmul(swp[:], pswapT[:], srcT[:, sl],
                                     start=True, stop=True)
                    tmp = ptmp.tile([128, TB], F32, name="tmp", tag="tmp")
                    nc.gpsimd.tensor_tensor(tmp[:], srcT[:, sl], cm[:], OP.mult)
                    tmp2 = ptmp.tile([128, TB], F32, name="tmp2", tag="tmp2")
                    nc.vector.tensor_tensor(tmp2[:], swp[:], sm[:], OP.mult)
                    nc.gpsimd.tensor_tensor(srcT[:, sl], tmp[:], tmp2[:], OP.add)
                # V transpose (+ones column) for this block's 4 k-tiles
                for lt in range(4):
                    kt = tb * 4 + lt
                    vtp = pvt.tile([128, 128], F32, name="vtp", tag="vtp")
                    nc.tensor.transpose(r32(vtp[:]),
                                        vT[:, kt * 128:(kt + 1) * 128],
                                        identR[:])
                    va0 = vap.tile([128, HDP1], BF16, name=f"va0_{kt}",
                                   tag=f"va0_{kt}")
                    nc.vector.memset(va0[:, HD:HDP1], 1.0)
                    nc.scalar.copy(va0[:, 0:HD], vtp[:, 0:HD])
                    va1 = vap.tile([128, HDP1], BF16, name=f"va1_{kt}",
                                   tag=f"va1_{kt}")
                    nc.vector.memset(va1[:, HD:HDP1], 1.0)
                    nc.vector.tensor_copy(va1[:, 0:HD], vtp[:, HD:128])
                    vaug[0][kt] = va0
                    vaug[1][kt] = va1
        es_v.close()

        # ======== Phase B: attention with per-batch pipelined epilogue
        es_b = ExitStack()
        big = es_b.enter_context(tc.tile_pool(name="big", bufs=1))
        out_n = big.tile([128, NT], F32R, name="out_n", tag="out_n")
        xio = big.tile([128, NT], BF16, name="xio", tag="xio")
        rsi = big.tile([33, QB], F32, name="rsi", tag="rsi")
        rsr = big.tile([33, QB], F32R, name="rsr", tag="rsr")
        nc.vector.memset(rsi[:], 1.0)
        prow = es_b.enter_context(tc.tile_pool(name="prow", bufs=2))
        so_rows = [None] * B
        iso_rows = [None] * B
        psSp = es_b.enter_context(tc.tile_pool(name="psS", bufs=3, space="PSUM"))
        pAcc = es_b.enter_context(tc.tile_pool(name="pAcc", bufs=1, space="PSUM"))
        pA = es_b.enter_context(tc.tile_pool(name="pA", bufs=3))
        pscl = es_b.enter_context(tc.tile_pool(name="pscl", bufs=2))
        pyq = es_b.enter_context(tc.tile_pool(name="pyq", bufs=2))
        pg = es_b.enter_context(tc.tile_pool(name="pg", bufs=2))
        pfin = es_b.enter_context(tc.tile_pool(name="pfin", bufs=2))
        pbi = es_b.enter_context(tc.tile_pool(name="pbi", bufs=1))
        bi_sb = [None] * NTB

        def post_ps():
            # post-work PSUM tiles share the psS pool (PSUM is fully budgeted)
            return psSp.tile([128, 2 * QB], F32, name="psS", tag="psS")

        def attn_batch(b, hooks=None):
            """Whole-batch attention as one flat software-pipelined stream:
            scores run 2 (qb, kl)-stages ahead of A@V, so neither the exp
            latency nor the per-qb epilogue ever starves the PE queue.
            hooks[i] is emitted after pipeline step i (mid-batch interleave
            of the other batch's post-processing)."""
            seq = [(qb, kl) for qb in range(NQB) for kl in range(4 * qb + 4)]
            n = len(seq)
            psA = {}
            psS_l = {}
            A_l = {}

            def scores(i):
                qb, kl = seq[i]
                # diag band tile v: queries < v*128 are fully masked — skip
                off = max(0, kl - 4 * qb) * 128
                qsl = slice(b * T + qb * QB + off, b * T + (qb + 1) * QB)
                kt = b * NKT + kl
                ksl = slice(kt * 128, (kt + 1) * 128)
                psS = psSp.tile([128, 2 * QB], F32, name="psS", tag="psS")
                for h in range(HPC):
                    hsl = slice(h * HD, (h + 1) * HD)
                    ssl = slice(h * QB + off, (h + 1) * QB)
                    nc.tensor.matmul(psS[:, ssl], kT[hsl, ksl], qT[hsl, qsl],
                                     start=True, stop=True)
                psS_l[i] = psS

            def do_exp(i):
                qb, kl = seq[i]
                A = pA.tile([128, 2 * QB], BF16, name="A", tag="A")
                nc.scalar.activation(A[:], psS_l.pop(i)[:], ACT.Exp,
                                     scale=1.0 / math.sqrt(HD))
                if kl >= 4 * qb:     # diagonal band: zero the future keys
                    v = kl - 4 * qb
                    nc.vector.tensor_tensor(
                        A[:], A[:], umask01[:, v * 2 * QB:(v + 1) * 2 * QB],
                        OP.mult)
                A_l[i] = A

            def av(i):
                qb, kl = seq[i]
                nkt = 4 * qb + 4
                if kl == 0:
                    psA[qb] = (pAcc.tile([128, QB], F32, name="psA0", tag="psA0"),
                               pAcc.tile([128, QB], F32, name="psA1", tag="psA1"))
                psA0, psA1 = psA[qb]
                kt = b * NKT + kl
                A = A_l.pop(i)
                off = max(0, kl - 4 * qb) * 128
                st, sp = kl == 0, kl == nkt - 1
                nc.tensor.matmul(psA0[0:HDP1, off:QB], vaug[0][kt][:],
                                 A[:, off:QB], start=st, stop=sp,
                                 skip_group_check=True)
                nc.tensor.matmul(psA1[0:HDP1, off:QB], vaug[1][kt][:],
                                 A[:, QB + off:2 * QB], start=st, stop=sp,
                                 skip_group_check=True)
                if sp:
                    epilogue(qb, psA.pop(qb))

            def epilogue(qb, psA01):
                psA0, psA1 = psA01
                qsl = slice(b * T + qb * QB, b * T + (qb + 1) * QB)
                # fused row-sums live at psA0[HD], psA1[HD]
                nc.vector.tensor_copy(rsi[0:1, :], psA0[HD:HDP1, :])
                nc.vector.tensor_copy(rsi[32:33, :], psA1[HD:HDP1, :])
                nc.vector.reciprocal(rsr[:], rsi[:])
                brs = post_ps()
                nc.tensor.matmul(brs[:, 0:QB], sel2[:], rsr[:], start=True,
                                 stop=True)
                brs_sb = pyq.tile([128, QB], F32, name="brs_sb", tag="brs_sb")
                nc.vector.tensor_copy(brs_sb[:], brs[:, 0:QB])
                nc.vector.tensor_tensor(out_n[0:HD, qsl], psA0[0:HD, :],
                                        brs_sb[0:HD, :], OP.mult)
                nc.vector.tensor_tensor(out_n[HD:128, qsl], psA1[0:HD, :],
                                        brs_sb[HD:128, :], OP.mult)

            scores(0)
            scores(1)
            do_exp(0)
            for i in range(2, n):
                scores(i)
                do_exp(i - 1)
                av(i - 2)
                if hooks and i in hooks:
                    hooks[i]()
            do_exp(n - 1)
            av(n - 2)
            av(n - 1)

        def amax_ar(b):
            am = pscl.tile([128, TTPB], F32, name="am", tag="am")
            for j in range(TTPB):
                gtt = b * TTPB + j
                tp = post_ps()
                nc.tensor.transpose(r32(tp[:, 0:128]),
                                    out_n[:, gtt * 128:(gtt + 1) * 128],
                                    identR[:])
                nc.vector.tensor_reduce(am[:, j:j + 1], tp[:, 0:128],
                                        mybir.AxisListType.X, OP.max,
                                        apply_absolute_value=True)
            nc.sync.dma_start(ar_in[b][:], am[:])
            nc.gpsimd.collective_compute(
                "AllReduce", OP.max, replica_groups=RG,
                ins=[ar_in[b][:].opt()], outs=[ar_out[b][:].opt()])

        def scale_rows(b):
            gmax = pscl.tile([128, TTPB], F32, name="gmax", tag="gmax")
            nc.sync.dma_start(gmax[:], ar_out[b][:])
            iso_p = pscl.tile([128, TTPB], F32R, name="iso_p", tag="iso_p")
            nc.vector.tensor_scalar(iso_p[:], gmax[:], 1e-5, 1.0 / 127.0,
                                    OP.add, OP.mult)
            so_p = pscl.tile([128, TTPB], F32R, name="so_p", tag="so_p")
            nc.vector.reciprocal(so_p[:], iso_p[:])
            tso = post_ps()
            nc.tensor.transpose(r32(tso[0:TTPB, 0:128]), so_p[:], identR[:])
            so_sq = pscl.tile([TTPB, 128], F32R, name="so_sq", tag="so_sq")
            nc.vector.tensor_copy(so_sq[:], tso[0:TTPB, 0:128])
            tiso = post_ps()
            nc.tensor.transpose(r32(tiso[0:TTPB, 0:128]), iso_p[:], identR[:])
            iso_sq = pscl.tile([TTPB, 128], F32R, name="iso_sq", tag="iso_sq")
            nc.vector.tensor_copy(iso_sq[:], tiso[0:TTPB, 0:128])
            so_rows[b] = prow.tile([1, T], F32R, name="so_row", tag="so_row")
            iso_rows[b] = prow.tile([1, T], F32R, name="iso_row", tag="iso_row")
            nc.sync.dma_start(so_rows[b][0:1, :], so_sq[:, :])
            nc.sync.dma_start(iso_rows[b][0:1, :], iso_sq[:, :])

        def quant_ag(seg):
            b, t0, ntok = AGSEG[seg]
            for tl in range(ntok // TB):
                sl = slice(t0 + tl * TB, t0 + (tl + 1) * TB)
                rsl = slice(t0 - b * T + tl * TB, t0 - b * T + (tl + 1) * TB)
                bso = post_ps()
                nc.tensor.matmul(bso[:, 0:QB], ones1[:], so_rows[b][:, rsl],
                                 start=True, stop=True)
                yq = pyq.tile([128, TB], F32, name="yq", tag="yq")
                nc.vector.tensor_tensor(yq[:], out_n[:, sl], bso[:, 0:QB],
                                        OP.mult)
                nc.vector.tensor_scalar(xio[:, sl], yq[:], RC, RC,
                                        OP.add, OP.subtract)
            nc.sync.dma_start(ag_in[seg][:], xio[:, t0:t0 + ntok])
            nc.gpsimd.collective_compute(
                "AllGather", OP.bypass, replica_groups=RG,
                ins=[ag_in[seg][:].opt()], outs=[ag_out[seg][:].opt()])

        def bi_all(b):
            # broadcast per-token 1/s_o for every block of batch b while the
            # PE is otherwise idle (AllReduce window) — off wo's critical path
            for tl in range(NQB):
                g = b * NQB + tl
                bi = post_ps()
                nc.tensor.matmul(bi[:, 0:QB], ones1[:],
                                 iso_rows[b][:, tl * TB:(tl + 1) * TB],
                                 start=True, stop=True)
                bi_sb[g] = pbi.tile([128, TB], F32, name=f"bi{g}", tag=f"bi{g}")
                nc.scalar.copy(bi_sb[g][:], bi[:, 0:QB])

        def wo(seg):
            b, t0, ntok = AGSEG[seg]
            for tl in range(ntok // TB):
                sl = slice(t0 + tl * TB, t0 + (tl + 1) * TB)
                g0 = (t0 - b * T) // TB + b * NQB + tl
                pw = post_ps()
                for i in range(8):
                    g = pg.tile([128, TB], BF16, name=f"g{i}", tag=f"g{i}")
                    nc.sync.dma_start(g[:], ag_out[seg][i * 128:(i + 1) * 128,
                                                       tl * TB:(tl + 1) * TB])
                    nc.tensor.matmul(pw[:, 0:QB], wo_sb[i][:], g[:],
                                     start=(i == 0), stop=(i == 7))
                fin = pfin.tile([128, TB], F32, name="fin", tag="fin")
                nc.vector.tensor_tensor(fin[:], pw[:, 0:QB], bi_sb[g0][:],
                                        OP.mult)
                nc.sync.dma_start(out[:, sl], fin[:])

        def post_b0():
            scale_rows(0)
            quant_ag(0)

        attn_batch(0)
        amax_ar(0)
        attn_batch(1, hooks={18: post_b0})
        amax_ar(1)
        bi_all(0)
        wo(0)
        scale_rows(1)
        bi_all(1)
        quant_ag(1)
        quant_ag(2)
        wo(1)
        wo(2)

        es_b.close()
        es_va.close()
        es_qk.close()

    return nc


_CACHE = {}


def kernel(x, cos, sin, wq_w, wk_w, wv_w, wo_w):
    x = np.asarray(x, np.float32)
    cos = np.asarray(cos, np.float32)   # [T, 32]
    sin = np.asarray(sin, np.float32)
    xf = np.ascontiguousarray(x.reshape(NT, D))

    amax = np.abs(xf).max(-1) + 1e-5
    sx = (127.0 / amax).astype(np.float32)
    isx = (amax / 127.0).astype(np.float32)
    sxp = np.ascontiguousarray(sx.reshape(NTT, 128).T)
    isx_bc = np.ascontiguousarray(np.broadcast_to(isx[None, :], (128, NT)))

    # RoPE maps from the provided cos/sin tables
    cm64 = np.repeat(cos.T, 2, axis=0)            # [64, T]
    sm64 = np.repeat(sin.T, 2, axis=0)
    # rows: [64 dims for head-even][64 dims for head-odd]; cols: [b0 | b1]
    cmap = np.tile(np.concatenate([cm64, cm64], axis=0), (1, B)).astype(np.float32)
    smap = np.tile(np.concatenate([sm64, sm64], axis=0), (1, B)).astype(np.float32)

    P = np.zeros((128, 128), np.float32)
    for j in range(64):
        P[2 * j, 2 * j + 1] = -1.0
        P[2 * j + 1, 2 * j] = 1.0
    pswapT = np.ascontiguousarray(P.T)
    kk = np.arange(128)[:, None]
    qq = np.arange(QB)[None, :]
    # keep-mask (1 = attend, 0 = future), duplicated for both head halves
    um1 = [((v * 128 + kk) <= qq).astype(np.float32) for v in range(4)]
    um01 = np.concatenate([np.concatenate([m, m], axis=1) for m in um1],
                          axis=1).astype(ml_dtypes.bfloat16)
    sel2 = np.zeros((33, 128), np.float32)
    sel2[0, 0:HD] = 1.0
    sel2[32, HD:128] = 1.0
    ones1 = np.ones((1, 128), np.float32)
    ident = np.eye(128, dtype=np.float32)
    identB = np.eye(128, dtype=np.float32).astype(ml_dtypes.bfloat16)

    wq_e, wk_e, wv_e, wo_e = (_quant_w(np.asarray(w, np.float32))
                              for w in (wq_w, wk_w, wv_w, wo_w))

    if "nc" not in _CACHE:
        nc0 = build_nc()
        nc0.finalize()
        _CACHE["nc"] = nc0
    nc = _CACHE["nc"]

    in_maps = []
    for c in range(N_CORES):
        hs = slice(c * DPC, (c + 1) * DPC)
        in_maps.append({
            "x": xf, "sxp": sxp, "isx": isx_bc,
            "wqT": np.ascontiguousarray(wq_e[hs, :].T),
            "wkT": np.ascontiguousarray(wk_e[hs, :].T),
            "wvT": np.ascontiguousarray(wv_e[hs, :].T),
            "woT": np.ascontiguousarray(wo_e[hs, :].T).astype(ml_dtypes.bfloat16),
            "cmap": cmap, "smap": smap, "pswapT": pswapT,
            "umask01": um01, "sel2": sel2, "ones1": ones1, "identR": ident,
            "identB": identB,
        })

    res = run_bass_kernel_spmd(nc, in_maps, core_ids=list(range(N_CORES)))
    outp = np.empty((NT, D), np.float32)
    for c in range(N_CORES):
        outp[:, c * DPC:(c + 1) * DPC] = res.results[c]["out"].T
    return outp.reshape(B, T, D)
